# revision 1
# baseline (speedup 1.0000x reference)
"""Trainium2 Bass kernel for nn_DeformableConvLSTMCell_33895881900284.

Full (unsharded) inputs in, full outputs out. Internally: data-parallel over
batch across 8 NeuronCores (8 batches per core), conv weights / gate params
replicated.

Math per the reference:
  outI  = conv3x3_same(inputs, wconvInput)
  g     = tanh(outI + conv3x3_same(hidden_prev, wconvHidden) + gateBias)
  gapI  = mean_hw(outI);  gapH = mean_hw(hidden_prev)          # [B, D]
  i/f/o = sigmoid(wx*gapI + wh*gapH + bias)                    # [B, D]
  tiled gate: value used at (b, h, w, c) is gate[(28*b + h) % 64, c]
  state  = f*state_prev + i*g;  hidden = o*tanh(state)

The (28*b+h)%64 scrambling makes gates cross-batch: each core computes its
local GAP columns, all cores AllGather them, and a per-core index-array input
drives an indirect-DMA gather of exactly the gate rows this core's outputs
need (the SPMD program stays identical across cores; only input data differs).

gapI never touches the conv output. By linearity, 784*gapI is a combination
of 9 masked pixel sums of the raw input (full sum, edge rows/cols, corners)
matmul'd with summed conv-weight taps. Stage A computes those masked sums
with tiny fp32 matmuls on natural-layout tiles (mask vectors as the moving
operand), so the AllGather fires ~50us into the kernel and the gate tables
are ready long before the first elementwise consumer. Stage B then runs a
single fused per-batch pipeline: PE transposes inputs/hidden/state_prev to
[channel, pixel] layout, 3x3 conv = 36 shifted float32r matmuls + a gateBias
identity-matmul accumulating in one PSUM bank per window, ACT applies tanh,
GpSimd applies the gates (broadcast via stride-0 APs), PE transposes the
results back, and DMA stores natural-layout outputs.

float32r = full-rate fp32 matmul with TF32-like operand rounding; operands
are produced by DVE copies with float32r output dtype.
"""
import numpy as np

import bass_rust
import concourse.bass as bass
import concourse.mybir as mybir
import concourse.tile as tile
from concourse.bass_utils import run_bass_kernel_spmd

F32 = mybir.dt.float32
F32R = mybir.dt.float32r
I32 = mybir.dt.int32
AF = mybir.ActivationFunctionType
ALU = mybir.AluOpType

N_CORES = 8
B, H, W, CIN, D = 64, 28, 28, 256, 256
BL = B // N_CORES          # local batches per core
PIX = H * W                # 784
PG = 112                   # pixels per transpose group (4 rows)
NPG = PIX // PG            # 7
PAD = 30                   # padded row/col length
XTLEN = PAD * PAD          # 900
NW = 2                     # windows per batch
WROWS = H // NW            # 14
WN = WROWS * W             # 392
NCC = CIN // 128           # 2 channel chunks
NDC = D // 128             # 2 output-channel chunks

# tap order t = 3*kh + kw ; dh = kh-1, dw = kw-1
TAPS = [(kh, kw) for kh in range(3) for kw in range(3)]

# ---------------------------------------------------------------------------
# walrus fixup: split semaphore waits that exceed the per-instruction budget
# (observed: Drain and Matmult accept only 1 semaphore wait each).
MAX_WAITS = 1


def _split_excess_sem_waits(nc):
    counter = [0]
    for fn in nc.m.functions:
        for bb in fn.blocks:
            insts = bb.instructions
            i = 0
            while i < len(insts):
                inst = insts[i]
                si = inst.sync_info
                if si is not None and si.on_wait and len(si.on_wait) > MAX_WAITS:
                    waits = list(si.on_wait)
                    excess = waits[:-MAX_WAITS]
                    keep = waits[-MAX_WAITS:]
                    new_insts = []
                    for j in range(0, len(excess), MAX_WAITS):
                        chunk = excess[j:j + MAX_WAITS]
                        noop = mybir.InstNoOp(
                            name=f"I-waitsplit-{counter[0]}", ins=[], outs=[])
                        counter[0] += 1
                        noop.engine = inst.engine
                        noop.sync_info = bass_rust.SyncInfo(
                            on_wait=chunk, on_update=[])
                        nc.register_instruction(noop)
                        new_insts.append(noop)
                    inst.sync_info = bass_rust.SyncInfo(
                        on_wait=keep, on_update=list(si.on_update))
                    insts[i:i] = new_insts
                    i += len(new_insts)
                i += 1
    return nc


# ---------------------------------------------------------------------------
def _r3(ap, w):
    """view flat free dim as (rows, w)"""
    return ap.rearrange("c (r w) -> c r w", w=w)


def build_nc():
    nc = bass.Bass("TRN2", target_bir_lowering=False, debug=False,
                   num_devices=N_CORES)

    dram = {}
    dram["inputs"] = nc.dram_tensor("inputs", [BL, PIX, CIN], F32,
                                    kind="ExternalInput").ap()
    dram["state_prev"] = nc.dram_tensor("state_prev", [BL, PIX, D], F32,
                                        kind="ExternalInput").ap()
    dram["hidden_prev"] = nc.dram_tensor("hidden_prev", [BL, PIX, D], F32,
                                         kind="ExternalInput").ap()
    dram["w1"] = nc.dram_tensor("wconvInput", [3, 3, CIN, D], F32,
                                kind="ExternalInput").ap()
    dram["w2"] = nc.dram_tensor("wconvHidden", [3, 3, D, D], F32,
                                kind="ExternalInput").ap()
    dram["gb"] = nc.dram_tensor("gateBias", [PIX, D], F32,
                                kind="ExternalInput").ap()
    dram["vec"] = {}
    for nm in ("wxi", "whi", "inputBias", "wxf", "whf", "forgetBias",
               "wxo", "who", "outputBias"):
        dram["vec"][nm] = nc.dram_tensor(nm, [D, 1], F32,
                                         kind="ExternalInput").ap()
    dram["ident"] = nc.dram_tensor("identity", [128, 128], F32,
                                   kind="ExternalInput").ap()
    dram["idx"] = nc.dram_tensor("gate_idx", [4, 224, 1], I32,
                                 kind="ExternalInput").ap()
    dram["masks"] = nc.dram_tensor("gap_masks", [NPG, PG, 9], F32,
                                   kind="ExternalInput").ap()
    dram["hidden"] = nc.dram_tensor("hidden", [BL, NDC, 128, PIX], F32,
                                    kind="ExternalOutput").ap()
    dram["state"] = nc.dram_tensor("state", [BL, NDC, 128, PIX], F32,
                                   kind="ExternalOutput").ap()
    dram["cc_in"] = nc.dram_tensor("cc_in", [32, 128], F32, kind="Internal").ap()
    dram["cc_out"] = nc.dram_tensor("cc_out", [N_CORES * 32, 128], F32,
                                    kind="Internal", addr_space="Shared").ap()

    ctx_mgr = nc.allow_low_precision("float32r operand rounding for PE")
    ctx_mgr.__enter__()
    with tile.TileContext(nc) as tc:
        _build_body(nc, tc, dram)
    ctx_mgr.__exit__(None, None, None)
    return nc


def _build_body(nc, tc, dram):
    from contextlib import ExitStack
    ctx = ExitStack()
    pool = lambda **kw: ctx.enter_context(tc.tile_pool(**kw))

    const = pool(name="const", bufs=1)
    wts = pool(name="wts", bufs=1)
    stage = pool(name="stage", bufs=4)
    natp = pool(name="natp", bufs=10)      # stage-A natural tiles
    xt_in = pool(name="xt_in", bufs=2)
    xt_hid = pool(name="xt_hid", bufs=2)
    xt_sp = pool(name="xt_sp", bufs=3)
    ew = pool(name="ew", bufs=3)
    outb = pool(name="outb", bufs=4)
    gsm = pool(name="gsm", bufs=1)
    gtmp = pool(name="gtmp", bufs=2)
    ps_conv = pool(name="ps_conv", bufs=3, space="PSUM")
    ps_tr = pool(name="ps_tr", bufs=3, space="PSUM")
    ps_gap = pool(name="ps_gap", bufs=1, space="PSUM")

    # ---- minimal constants needed by stage A ----
    ident = const.tile([128, 128], F32, tag="ident")
    nc.sync.dma_start(ident[:], dram["ident"][:])
    masks = []
    for pg in range(NPG):
        m = const.tile([PG, 9], F32, tag=f"mask{pg}", name=f"mask{pg}")
        nc.sync.dma_start(m[:], dram["masks"][pg])
        masks.append(m)
    idx_sb = []
    for g4 in range(4):
        halves = []
        for hf in range(2):
            t = const.tile([PG, 1], I32, tag=f"idx{g4}_{hf}")
            nc.sync.dma_start(t[:], dram["idx"][g4, hf * PG:(hf + 1) * PG, :])
            halves.append(t)
        idx_sb.append(halves)

    raw = [gsm.tile([128, 9 * BL], F32R, tag=f"raw{cc}", name=f"raw{cc}")
           for cc in range(NCC)]
    gapH = [gsm.tile([128, BL], F32, tag=f"gapH{cc}", name=f"gapHs{cc}")
            for cc in range(NCC)]

    # ============ stage B: fused per-batch pipeline =========================
    from collections import deque
    tqueue = deque()   # pending transpose-emitter closures (next batch)

    def drain_tq(n):
        for _ in range(min(n, len(tqueue))):
            tqueue.popleft()()

    def queue_transposed(j, dsrc, xpool, tagbase, dtype, padded):
        """queue DMA+transpose+copy work building [128, 900|784] per cc."""
        tiles = []
        for cc in range(NCC):
            xlen = XTLEN if padded else PIX
            xt = xpool.tile([128, xlen], dtype, tag=f"{tagbase}{cc}",
                            name=f"{tagbase}{cc}_{j}")
            if padded:
                x3 = _r3(xt[:].bitcast(F32), PAD)
                nc.gpsimd.memset(x3[:, 0:1, :], 0.0)
                nc.gpsimd.memset(x3[:, PAD - 1:PAD, :], 0.0)
                nc.gpsimd.memset(x3[:, 1:PAD - 1, 0:1], 0.0)
                nc.gpsimd.memset(x3[:, 1:PAD - 1, PAD - 1:PAD], 0.0)
            tiles.append(xt)

        def emit_group(g7):
            nat = stage.tile([128, 256], F32, tag="natload", name="natload")
            nc.sync.dma_start(nat[0:PG, :], dsrc[j, g7 * PG:(g7 + 1) * PG, :])
            for cc in range(NCC):
                pt = ps_tr.tile([128, PG], F32, tag="ptr", name="pt_tr")
                nc.tensor.transpose(pt[:], nat[0:PG, cc * 128:(cc + 1) * 128],
                                    ident[0:PG, 0:PG])
                if padded:
                    dst = _r3(tiles[cc][:], PAD)[:, 1 + 4 * g7:1 + 4 * g7 + 4,
                                                 1:29]
                else:
                    dst = _r3(tiles[cc][:, g7 * PG:(g7 + 1) * PG], W)
                nc.vector.tensor_copy(dst,
                                      pt[:].rearrange("c (r w) -> c r w", w=W))

        for g7 in range(NPG):
            tqueue.append(lambda g7=g7: emit_group(g7))
        return tiles

    xbufs = {}

    def queue_batch_inputs(j):
        xbufs[j] = (
            queue_transposed(j, dram["inputs"], xt_in, "xin", F32R, True),
            queue_transposed(j, dram["hidden_prev"], xt_hid, "xhid", F32R, True),
            queue_transposed(j, dram["state_prev"], xt_sp, "xsp", F32, False),
        )

    def emit_conv_window(j, wi, dc):
        """conv+bias -> PSUM -> tanh -> gt tile; returns gt"""
        xin, xhid, _ = xbufs[j]
        h0 = 1 + wi * WROWS
        base = (h0 - 1) * W
        p = ps_conv.tile([128, WN], F32, tag="pconv", name="pconv")
        p3 = _r3(p[:], W)
        nc.tensor.matmul(p[:], ident_r[:], gbias[dc][:, base:base + WN],
                         start=True, stop=False)
        for conv, xbuf in ((0, xin), (1, xhid)):
            for t, (kh, kw) in enumerate(TAPS):
                dh, dwid = kh - 1, kw - 1
                for cc in range(NCC):
                    rhs = _r3(xbuf[cc][:], PAD)[
                        :, h0 + dh:h0 + dh + WROWS, 1 + dwid:1 + dwid + W]
                    last = (conv == 1 and t == 8 and cc == NCC - 1)
                    nc.tensor.matmul(
                        p3, wblk(conv, t, cc)[:, dc * 128:(dc + 1) * 128],
                        rhs, start=False, stop=last)
        gt = ew.tile([128, WN], F32, tag="gt", bufs=6, name="gt")
        nc.scalar.activation(gt[:], p[:], AF.Tanh)
        return gt

    def emit_elementwise(j, wi, dc, gt, stT, hidT, gates):
        _, _, xsp = xbufs[j]
        h0 = 1 + wi * WROWS
        base = (h0 - 1) * W
        t0 = j * H + (h0 - 1)

        def gw(gate):
            return gates[gate][dc][:, t0:t0 + WROWS].to_broadcast(
                [128, WROWS, W])

        sp3 = _r3(xsp[dc][:, base:base + WN], W)
        g3 = _r3(gt[:], W)
        st3 = _r3(stT[dc][:, base:base + WN], W)
        hd3 = _r3(hidT[dc][:, base:base + WN], W)
        s1 = ew.tile([128, WN], F32, tag="s1", name="s1")
        nc.gpsimd.tensor_tensor(out=_r3(s1[:], W), in0=sp3, in1=gw("f"),
                                op=ALU.mult)
        s2 = ew.tile([128, WN], F32, tag="s2", name="s2")
        nc.gpsimd.tensor_tensor(out=_r3(s2[:], W), in0=g3, in1=gw("i"),
                                op=ALU.mult)
        nc.vector.tensor_tensor(out=st3, in0=_r3(s1[:], W), in1=_r3(s2[:], W),
                                op=ALU.add)
        th = ew.tile([128, WN], F32, tag="th", name="th")
        nc.scalar.activation(th[:], stT[dc][:, base:base + WN], AF.Tanh)
        nc.gpsimd.tensor_tensor(out=hd3, in0=_r3(th[:], W), in1=gw("o"),
                                op=ALU.mult)

    def emit_store(j, stT, hidT):
        # outputs leave the chip transposed ([dc, 128, pix]); the host
        # reassembles to NHWC during unsharding.
        for dname, buf in (("state", stT), ("hidden", hidT)):
            for dc in range(NDC):
                nc.scalar.dma_start(dram[dname][j, dc], buf[dc][:])

    def out_tiles(j):
        stT = [outb.tile([128, PIX], F32, tag="stT", name=f"stT{j}_{dc}")
               for dc in range(NDC)]
        hidT = [outb.tile([128, PIX], F32, tag="hidT", name=f"hidT{j}_{dc}")
                for dc in range(NDC)]
        return stT, hidT

    queue_batch_inputs(0)
    queue_batch_inputs(1)

    # ============ stage A: masked pixel sums via fp32 matmuls ==============
    # lhsT = mask vectors [112, 9] (9-column weight load: cheap),
    # rhs = natural-layout tiles [112, 256]; out = RAW^T [9, 256] accumulated
    # over the 7 pixel groups, then transposed into [c, 9]/[c, 1] form.
    # ---- remaining constants (overlap the stage-A matmul stream) ----
    ident_r = const.tile([128, 128], F32R, tag="ident_r")
    nc.vector.tensor_copy(ident_r[:], ident[:])
    vecs = {}
    for nm in dram["vec"]:
        t = const.tile([128, NDC], F32, tag=f"vec_{nm}")
        for c in range(NDC):
            nc.scalar.dma_start(t[:, c:c + 1],
                                dram["vec"][nm][c * 128:(c + 1) * 128, :])
        if nm.startswith("wx") or nm.startswith("wh"):
            nc.vector.tensor_scalar_mul(t[:], t[:], 1.0 / PIX)
        vecs[nm] = t

    wconv = wts.tile([128, 2 * 9 * NCC * 256], F32R, tag="wconv")

    def wblk(conv, t, cc):
        off = ((conv * 9 + t) * NCC + cc) * 256
        return wconv[:, off:off + 256]

    for conv, dw in ((0, dram["w1"]), (1, dram["w2"])):
        for t, (kh, kw) in enumerate(TAPS):
            for cc in range(NCC):
                ws = stage.tile([128, 256], F32, tag="wstage")
                nc.scalar.dma_start(ws[:], dw[kh, kw, cc * 128:(cc + 1) * 128, :])
                nc.scalar.copy(wblk(conv, t, cc), ws[:])

    # gateBias transposed to [128 d, PIX] per dc, f32r (conv-PSUM accumuland)
    gbias = [const.tile([128, PIX], F32R, tag=f"gbias{dc}", name=f"gbias{dc}")
             for dc in range(NDC)]
    for g7 in range(NPG):
        nat = stage.tile([128, 256], F32, tag="natload")
        nc.scalar.dma_start(nat[0:PG, :], dram["gb"][g7 * PG:(g7 + 1) * PG, :])
        for dc in range(NDC):
            pt = ps_tr.tile([128, PG], F32, tag="ptr")
            nc.tensor.transpose(pt[:], nat[0:PG, dc * 128:(dc + 1) * 128],
                                ident[0:PG, 0:PG])
            nc.vector.tensor_copy(gbias[dc][:, g7 * PG:(g7 + 1) * PG], pt[:])


    gt0 = {}
    masks_r = []
    for pg in range(NPG):
        mr = const.tile([PG, 9], F32R, tag=f"maskr{pg}", name=f"maskr{pg}")
        nc.vector.tensor_copy(mr[:], masks[pg][:])
        masks_r.append(mr)
    for j in range(BL):
        for tensor, dsrc in (("in", dram["inputs"]), ("hid", dram["hidden_prev"])):
            p9 = ps_gap.tile([9, 256], F32, tag="rawT")
            for pg in range(NPG):
                nat = natp.tile([PG, 256], F32, tag="nat",
                                name=f"nat{tensor}{j}_{pg}")
                nc.sync.dma_start(nat[:], dsrc[j, pg * PG:(pg + 1) * PG, :])
                natr = natp.tile([PG, 256], F32R, tag="natr",
                                 name=f"natr{tensor}{j}_{pg}")
                nc.vector.tensor_copy(natr[:], nat[:])
                nc.tensor.matmul(p9[:], masks_r[pg][:], natr[:],
                                 start=(pg == 0), stop=(pg == NPG - 1))
            rt = gtmp.tile([9, 256], F32, tag="rawT_sb")
            nc.vector.tensor_copy(rt[:], p9[:])
            for cc in range(NCC):
                cs = slice(cc * 128, (cc + 1) * 128)
                pt = ps_tr.tile([128, 9], F32, tag="ptr")
                nc.tensor.transpose(pt[:], rt[:, cs], ident[0:9, 0:9])
                if tensor == "in":
                    nc.vector.tensor_copy(
                        _r3(raw[cc][:], BL)[:, :, j:j + 1],
                        pt[:].rearrange("c (n o) -> c n o", o=1))
                else:
                    nc.vector.tensor_copy(gapH[cc][:, j:j + 1], pt[:, 0:1])
            drain_tq(3)
        if j == 5:
            gt0[(0, 0)] = emit_conv_window(0, 0, 0)
            gt0[(0, 1)] = emit_conv_window(0, 0, 1)
        elif j == 6:
            gt0[(1, 0)] = emit_conv_window(0, 1, 0)
        elif j == 7:
            gt0[(1, 1)] = emit_conv_window(0, 1, 1)

    drain_tq(len(tqueue))

    # combined A-tiles for gapI (conv1 weights); group order:
    # S, Rf, Rl, Cf, Cl, K00, K0L, KL0, KLL (natural coords: Rf=row0, Cl=col27)
    a_r = wts.tile([128, NCC * 9 * 256], F32R, tag="a_r")

    def ablk(cc, g):
        off = (cc * 9 + g) * 256
        return a_r[:, off:off + 256]

    for cc in range(NCC):
        nc.vector.tensor_copy(ablk(cc, 0), wblk(0, 0, cc))
        for t in range(1, 9):
            nc.vector.tensor_tensor(out=ablk(cc, 0), in0=ablk(cc, 0),
                                    in1=wblk(0, t, cc), op=ALU.add)
        for g, taps in ((1, [6, 7, 8]), (2, [0, 1, 2]),
                        (3, [2, 5, 8]), (4, [0, 3, 6])):
            nc.vector.tensor_copy(ablk(cc, g), wblk(0, taps[0], cc))
            for t in taps[1:]:
                nc.vector.tensor_tensor(out=ablk(cc, g), in0=ablk(cc, g),
                                        in1=wblk(0, t, cc), op=ALU.add)
            nc.vector.tensor_scalar_mul(ablk(cc, g), ablk(cc, g), -1.0)
        for g, t in ((5, 8), (6, 6), (7, 2), (8, 0)):
            nc.vector.tensor_copy(ablk(cc, g), wblk(0, t, cc))


    # ---- gapI combine + staging + AllGather launch ----
    gap_ps = ps_gap.tile([8, 256], F32, tag="gapI")
    for cc in range(NCC):
        for g in range(9):
            nc.tensor.matmul(gap_ps[:], raw[cc][:, g * BL:(g + 1) * BL],
                             ablk(cc, g),
                             start=(cc == 0 and g == 0),
                             stop=(cc == NCC - 1 and g == 8))
    gapI_sb = gsm.tile([8, 256], F32, tag="gapI_sb")
    nc.vector.tensor_copy(gapI_sb[:], gap_ps[:])
    nc.sync.dma_start(dram["cc_in"][0:8, :], gapI_sb[:, 0:128])
    nc.sync.dma_start(dram["cc_in"][8:16, :], gapI_sb[:, 128:256])
    for cc in range(NCC):
        pt = ps_gap.tile([8, 128], F32, tag="rawT")
        nc.tensor.transpose(pt[:], gapH[cc][:], ident[:])
        hs = gsm.tile([8, 128], F32, tag=f"gapH_sb{cc}", name=f"gapHsb{cc}")
        nc.vector.tensor_copy(hs[:], pt[:])
        nc.sync.dma_start(dram["cc_in"][16 + 8 * cc:24 + 8 * cc, :], hs[:])

    nc.gpsimd.collective_compute(
        "AllGather", ALU.bypass, replica_groups=[list(range(N_CORES))],
        ins=[dram["cc_in"][:]], outs=[dram["cc_out"][:]])

    # batch 0: inputs + convs first (no gate dependency), gates next,
    # elementwise afterwards - keeps the PE stream from head-of-line
    # blocking on the AllGather.

    # ---- gather + gate tables (waits on AllGather, off the conv path) ----
    sel = [gsm.tile([128, 224], F32, tag=f"sel{g4}", name=f"sel{g4}")
           for g4 in range(4)]
    for g4 in range(4):
        for hf in range(2):
            gtile = stage.tile([PG, 128], F32, tag="gath", name="gath")
            nc.gpsimd.indirect_dma_start(
                out=gtile[:], out_offset=None, in_=dram["cc_out"][:],
                in_offset=bass.IndirectOffsetOnAxis(ap=idx_sb[g4][hf][:, :1],
                                                    axis=0))
            pt = ps_tr.tile([128, PG], F32, tag="ptr", name="pt_gath")
            nc.tensor.transpose(pt[:], gtile[:], ident[0:PG, 0:PG])
            nc.vector.tensor_copy(sel[g4][:, hf * PG:(hf + 1) * PG], pt[:])

    gates = {}
    for gate, wx, wh, bi in (("i", "wxi", "whi", "inputBias"),
                             ("f", "wxf", "whf", "forgetBias"),
                             ("o", "wxo", "who", "outputBias")):
        per_dc = []
        for dc in range(NDC):
            t1 = gtmp.tile([128, 224], F32, tag="gm1", name="gm1")
            nc.vector.tensor_scalar_mul(t1[:], sel[dc][:],
                                        vecs[wx][:, dc:dc + 1])
            t2 = gtmp.tile([128, 224], F32, tag="gm2", name="gm2")
            nc.vector.tensor_scalar_mul(t2[:], sel[2 + dc][:],
                                        vecs[wh][:, dc:dc + 1])
            nc.vector.tensor_tensor(out=t1[:], in0=t1[:], in1=t2[:], op=ALU.add)
            gt = gsm.tile([128, 224], F32, tag=f"gate_{gate}{dc}",
                          name=f"gate_{gate}{dc}")
            nc.scalar.activation(gt[:], t1[:], AF.Sigmoid,
                                 bias=vecs[bi][:, dc:dc + 1])
            per_dc.append(gt)
        gates[gate] = per_dc

    # batch 0 elementwise + store
    stT, hidT = out_tiles(0)
    for wi in range(NW):
        for dc in range(NDC):
            emit_elementwise(0, wi, dc, gt0[(wi, dc)], stT, hidT, gates)
            drain_tq(3)
    emit_store(0, stT, hidT)

    # batches 1..7
    for j in range(1, BL):
        if j + 1 < BL:
            queue_batch_inputs(j + 1)
        drain_tq(6)
        stT, hidT = out_tiles(j)
        for wi in range(NW):
            for dc in range(NDC):
                gt = emit_conv_window(j, wi, dc)
                emit_elementwise(j, wi, dc, gt, stT, hidT, gates)
                drain_tq(4 if j + 1 < BL else len(tqueue))
        emit_store(j, stT, hidT)
    drain_tq(len(tqueue))

    ctx.close()


# ---------------------------------------------------------------------------
_NC_CACHE = None


def _get_nc():
    global _NC_CACHE
    if _NC_CACHE is None:
        nc = build_nc()
        _split_excess_sem_waits(nc)
        _NC_CACHE = nc
    return _NC_CACHE


def _gate_idx(core):
    idx = np.empty((4, 224, 1), np.int32)
    for j in range(BL):
        for hh in range(H):
            t = j * H + hh
            sel_b = (H * (BL * core + j) + hh) % B
            cp, bp = sel_b // BL, sel_b % BL
            for g in range(4):
                idx[g, t, 0] = cp * 32 + g * 8 + bp
    return idx


def _gap_masks():
    m = np.zeros((PIX, 9), np.float32)
    hw = np.arange(PIX)
    r, c = hw // W, hw % W
    m[:, 0] = 1.0
    m[r == 0, 1] = 1.0
    m[r == H - 1, 2] = 1.0
    m[c == 0, 3] = 1.0
    m[c == W - 1, 4] = 1.0
    m[(r == 0) & (c == 0), 5] = 1.0
    m[(r == 0) & (c == W - 1), 6] = 1.0
    m[(r == H - 1) & (c == 0), 7] = 1.0
    m[(r == H - 1) & (c == W - 1), 8] = 1.0
    return m.reshape(NPG, PG, 9)


def _make_in_maps(inputs):
    f32 = np.float32
    shared = {
        "wconvInput": np.ascontiguousarray(inputs["wconvInput"], dtype=f32),
        "wconvHidden": np.ascontiguousarray(inputs["wconvHidden"], dtype=f32),
        "gateBias": np.ascontiguousarray(inputs["gateBias"],
                                         dtype=f32).reshape(PIX, D),
        "identity": np.eye(128, dtype=f32),
        "gap_masks": _gap_masks(),
    }
    for nm in ("wxi", "whi", "inputBias", "wxf", "whf", "forgetBias",
               "wxo", "who", "outputBias"):
        shared[nm] = np.ascontiguousarray(inputs[nm], dtype=f32).reshape(D, 1)

    xin = np.ascontiguousarray(inputs["inputs"], dtype=f32).reshape(B, PIX, CIN)
    xsp = np.ascontiguousarray(inputs["state_prev"], dtype=f32).reshape(B, PIX, D)
    xhp = np.ascontiguousarray(inputs["hidden_prev"], dtype=f32).reshape(B, PIX, D)

    in_maps = []
    for k in range(N_CORES):
        sl = slice(k * BL, (k + 1) * BL)
        m = dict(shared)
        m["inputs"] = xin[sl]
        m["state_prev"] = xsp[sl]
        m["hidden_prev"] = xhp[sl]
        m["gate_idx"] = _gate_idx(k)
        in_maps.append(m)
    return in_maps


def kernel(**inputs):
    nc = _get_nc()
    in_maps = _make_in_maps(inputs)
    res = run_bass_kernel_spmd(nc, in_maps, core_ids=list(range(N_CORES)))

    def unshard(name):
        # per-core outputs are [BL, NDC, 128, PIX] (channel-major); restore NHWC
        full = np.concatenate([res.results[k][name] for k in range(N_CORES)],
                              axis=0)
        return np.ascontiguousarray(full.transpose(0, 3, 1, 2)).reshape(
            B, H, W, D)

    return unshard("hidden"), unshard("state")



# revision 9
# speedup vs baseline: 1.3963x; 1.3963x over previous
"""Trainium2 Bass kernel for nn_DeformableConvLSTMCell_33895881900284.

Full (unsharded) inputs in, full outputs out. Data-parallel over batch across
8 NeuronCores (8 batches per core), conv weights / gate params replicated.

Math per the reference:
  outI  = conv3x3_same(inputs, wconvInput)
  g     = tanh(outI + conv3x3_same(hidden_prev, wconvHidden) + gateBias)
  gapI  = mean_hw(outI);  gapH = mean_hw(hidden_prev)          # [B, D]
  i/f/o = sigmoid(wx*gapI + wh*gapH + bias)                    # [B, D]
  tiled gate: value used at (b, h, w, c) is gate[(28*b + h) % 64, c]
  state  = f*state_prev + i*g;  hidden = o*tanh(state)

v2 design (vs the f32r baseline):
  * Host pre-transposes inputs/hidden/state to channel-major [BL, cc, 128,
    784] (layout-only, like the baseline's host-side output untranspose), so
    the kernel does no PE transposes and no DVE layout copies.
  * The whole conv path runs in bf16: weights/inputs/gateBias are cast on
    the host, conv = 36 shifted bf16 matmuls + a bias identity-matmul per
    392-pixel window accumulating in fp32 PSUM. bf16 enables the PE's Fast
    Weight Load path (fp32r disables it), roughly halving per-matmul cost.
  * gapI never touches the conv output: by linearity 784*gapI is a
    combination of 9 masked pixel sums of the raw input with host-folded
    A-matrices (sums of conv taps). The masked sums (full/edge/corner) are
    free-dim DVE reductions over the staged channel-major tiles, so the
    AllGather of gate drivers fires ~45us in, long before the first
    elementwise consumer.
  * The (28*b+h)%64 gate scrambling makes gates cross-batch: cores AllGather
    their local GAP columns and a per-core index-array input drives an
    indirect-DMA gather of exactly the gate rows this core needs (the SPMD
    program stays identical across cores; only input data differs).
"""
import numpy as np
import ml_dtypes

import bass_rust
import concourse.bass as bass
import concourse.mybir as mybir
import concourse.tile as tile
from concourse.bass_utils import run_bass_kernel_spmd

F32 = mybir.dt.float32
BF16 = mybir.dt.bfloat16
I32 = mybir.dt.int32
AF = mybir.ActivationFunctionType
ALU = mybir.AluOpType

N_CORES = 8
B, H, W, CIN, D = 64, 28, 28, 256, 256
BL = B // N_CORES          # local batches per core
PIX = H * W                # 784
PAD = 30                   # padded row/col length
XTLEN = PAD * PAD          # 900
NW = 2                     # windows per batch
WROWS = H // NW            # 14
WN = WROWS * W             # 392
NCC = CIN // 128           # 2 input-channel chunks
NDC = D // 128             # 2 output-channel chunks

# tap order t = 3*kh + kw ; dh = kh-1, dw = kw-1
TAPS = [(kh, kw) for kh in range(3) for kw in range(3)]

# ---------------------------------------------------------------------------
# walrus fixup: split semaphore waits that exceed the per-instruction budget
MAX_WAITS = 1


def _split_excess_sem_waits(nc):
    counter = [0]
    for fn in nc.m.functions:
        for bb in fn.blocks:
            insts = bb.instructions
            i = 0
            while i < len(insts):
                inst = insts[i]
                si = inst.sync_info
                if si is not None and si.on_wait and len(si.on_wait) > MAX_WAITS:
                    waits = list(si.on_wait)
                    excess = waits[:-MAX_WAITS]
                    keep = waits[-MAX_WAITS:]
                    new_insts = []
                    for j in range(0, len(excess), MAX_WAITS):
                        chunk = excess[j:j + MAX_WAITS]
                        noop = mybir.InstNoOp(
                            name=f"I-waitsplit-{counter[0]}", ins=[], outs=[])
                        counter[0] += 1
                        noop.engine = inst.engine
                        noop.sync_info = bass_rust.SyncInfo(
                            on_wait=chunk, on_update=[])
                        nc.register_instruction(noop)
                        new_insts.append(noop)
                    inst.sync_info = bass_rust.SyncInfo(
                        on_wait=keep, on_update=list(si.on_update))
                    insts[i:i] = new_insts
                    i += len(new_insts)
                i += 1
    return nc


# ---------------------------------------------------------------------------
def _r3(ap, w):
    """view flat free dim as (rows, w)"""
    return ap.rearrange("c (r w) -> c r w", w=w)


def build_nc():
    nc = bass.Bass("TRN2", target_bir_lowering=False, debug=False,
                   num_devices=N_CORES)

    dram = {}
    dram["xin"] = nc.dram_tensor("inputs_t", [BL, NCC, 128, PIX], BF16,
                                 kind="ExternalInput").ap()
    dram["xhid"] = nc.dram_tensor("hidden_t", [BL, NCC, 128, PIX], BF16,
                                  kind="ExternalInput").ap()
    dram["xsp"] = nc.dram_tensor("state_t", [BL, NCC, 128, PIX], F32,
                                 kind="ExternalInput").ap()
    dram["wconv"] = nc.dram_tensor("wconv_bf", [2, 9, NCC, 128, D], BF16,
                                   kind="ExternalInput").ap()
    dram["afold"] = nc.dram_tensor("a_fold", [NCC, 9, 128, D], BF16,
                                   kind="ExternalInput").ap()
    dram["gb"] = nc.dram_tensor("gbias_t", [NDC, 128, PIX], BF16,
                                kind="ExternalInput").ap()
    dram["vec"] = {}
    for nm in ("wxi", "whi", "inputBias", "wxf", "whf", "forgetBias",
               "wxo", "who", "outputBias"):
        dram["vec"][nm] = nc.dram_tensor(nm, [128, NDC], F32,
                                         kind="ExternalInput").ap()
    dram["ident"] = nc.dram_tensor("identity", [128, 128], F32,
                                   kind="ExternalInput").ap()
    dram["identbf"] = nc.dram_tensor("identity_bf", [128, 128], BF16,
                                     kind="ExternalInput").ap()
    dram["idx"] = nc.dram_tensor("gate_idx", [4, 224, 1], I32,
                                 kind="ExternalInput").ap()
    dram["hidden"] = nc.dram_tensor("hidden", [BL, NDC, 128, PIX], F32,
                                    kind="ExternalOutput").ap()
    dram["state"] = nc.dram_tensor("state", [BL, NDC, 128, PIX], F32,
                                   kind="ExternalOutput").ap()
    dram["cc_in"] = nc.dram_tensor("cc_in", [32, 128], F32, kind="Internal").ap()
    dram["cc_out"] = nc.dram_tensor("cc_out", [N_CORES * 32, 128], F32,
                                    kind="Internal", addr_space="Shared").ap()

    ctx_mgr = nc.allow_low_precision("bf16 conv path")
    ctx_mgr.__enter__()
    with tile.TileContext(nc) as tc:
        _build_body(nc, tc, dram)
    ctx_mgr.__exit__(None, None, None)
    return nc


def _build_body(nc, tc, dram):
    from contextlib import ExitStack
    ctx = ExitStack()
    pool = lambda **kw: ctx.enter_context(tc.tile_pool(**kw))

    const = pool(name="const", bufs=1)
    stg = pool(name="stg", bufs=4)         # [128, 784] bf16 staging
    xspp = pool(name="xspp", bufs=6)       # [128, 784] f32 state tiles
    gtp = pool(name="gtp", bufs=12)        # [128, 392] f32 tanh-conv tiles
    outb = pool(name="outb", bufs=4)       # [128, 784] f32 out tiles (per tag)
    ew = pool(name="ew", bufs=3)           # [128, 392] f32 scratch (per tag)
    gtmp = pool(name="gtmp", bufs=2)
    gath = pool(name="gath", bufs=2)
    ps_conv = pool(name="ps_conv", bufs=6, space="PSUM")
    ps_gap = pool(name="ps_gap", bufs=1, space="PSUM")
    ps_tr = pool(name="ps_tr", bufs=1, space="PSUM")

    # ---- constants (scalar/ACT DMA ring; order = need order) ----
    identbf = const.tile([128, 128], BF16, tag="identbf")
    nc.scalar.dma_start(identbf[:], dram["identbf"][:])

    wconv = const.tile([128, 2 * 9 * NCC * D], BF16, tag="wconv")

    def wblk(conv, t, cc):
        off = ((conv * 9 + t) * NCC + cc) * D
        return wconv[:, off:off + D]

    # dram [2,9,NCC,128,D] -> SBUF [128, (2,9,NCC,D)] in one strided DMA
    nc.scalar.dma_start(
        wconv[:].rearrange("p (a t c n) -> p a t c n", a=2, t=9, c=NCC),
        dram["wconv"][:].rearrange("a t c p n -> p a t c n"))

    gbias = [const.tile([128, PIX], BF16, tag=f"gbias{dc}", name=f"gbias{dc}")
             for dc in range(NDC)]
    for dc in range(NDC):
        nc.scalar.dma_start(gbias[dc][:], dram["gb"][dc])

    afold = const.tile([128, NCC * 9 * D], BF16, tag="afold")

    def ablk(cc, g):
        off = (cc * 9 + g) * D
        return afold[:, off:off + D]

    nc.scalar.dma_start(
        afold[:].rearrange("p (c g n) -> p c g n", c=NCC, g=9),
        dram["afold"][:].rearrange("c g p n -> p c g n"))

    idx_sb = []
    for g4 in range(4):
        halves = []
        for hf in range(2):
            t = const.tile([112, 1], I32, tag=f"idx{g4}_{hf}")
            nc.scalar.dma_start(t[:], dram["idx"][g4, hf * 112:(hf + 1) * 112, :])
            halves.append(t)
        idx_sb.append(halves)

    vecs = {}
    for nm in dram["vec"]:
        t = const.tile([128, NDC], F32, tag=f"vec_{nm}")
        nc.scalar.dma_start(t[:], dram["vec"][nm][:])
        vecs[nm] = t

    ident = const.tile([128, 128], F32, tag="ident")
    nc.scalar.dma_start(ident[:], dram["ident"][:])

    # ---- padded input tiles: one fixed slot per (batch, tensor, cc) ----
    # memset zeroes the whole tile once; only the 28x28 interior is
    # rewritten per run, so the pad border stays zero.
    xt = {}       # xt[(j, tensor, cc)] -> [128, 900] bf16
    for j in range(BL):
        for tn in ("in", "hid"):
            for cc in range(NCC):
                t = const.tile([128, XTLEN], BF16, tag=f"x{tn}{j}_{cc}",
                               name=f"x{tn}{j}_{cc}")
                nc.gpsimd.memset(t[:], 0.0)
                xt[(j, tn, cc)] = t

    # masked-sum accumulators (columns written per batch)
    rawI = [const.tile([128, 9 * BL], BF16, tag=f"rawI{cc}", name=f"rawI{cc}")
            for cc in range(NCC)]
    rawH = [const.tile([128, BL], F32, tag=f"rawH{cc}", name=f"rawH{cc}")
            for cc in range(NCC)]

    # ---- per-batch input loading (sync/SP DMA ring) ----
    stg_tiles = {}

    def emit_load(j):
        for tn, dsrc in (("in", dram["xin"]), ("hid", dram["xhid"])):
            for cc in range(NCC):
                s = stg.tile([128, PIX], BF16, tag="stg",
                             name=f"stg{tn}{j}_{cc}")
                nc.sync.dma_start(s[:], dsrc[j, cc])
                stg_tiles[(j, tn, cc)] = s

    xsp_tiles = {}

    def emit_load_state(j):
        ts = []
        for cc in range(NCC):
            s = xspp.tile([128, PIX], F32, tag="xsp", name=f"xsp{j}_{cc}")
            nc.sync.dma_start(s[:], dram["xsp"][j, cc])
            ts.append(s)
        xsp_tiles[j] = ts

    # ---- stage: pad-copy (scalar) + masked-sum reductions (vector) ----
    def emit_stage(j):
        for tn in ("in", "hid"):
            for cc in range(NCC):
                s = stg_tiles.pop((j, tn, cc))
                s3 = _r3(s[:], W)                       # [128, 28, 28]
                dst = _r3(xt[(j, tn, cc)][:], PAD)[:, 1:29, 1:29]
                nc.scalar.copy(dst, s3)
                if tn == "hid":
                    # gapH: full pixel sum only
                    nc.vector.tensor_reduce(
                        rawH[cc][:, j:j + 1], s[:], mybir.AxisListType.X,
                        ALU.add)
                else:
                    rv = rawI[cc][:].rearrange("c (g b) -> c g b", b=BL)
                    # group order: S, Rf(row0), Rl(row27), Cf(col0),
                    # Cl(col27), K00, K0L, KL0, KLL
                    nc.vector.tensor_reduce(
                        rv[:, 0, j:j + 1], s[:], mybir.AxisListType.X, ALU.add)
                    nc.vector.tensor_reduce(
                        rv[:, 1, j:j + 1], s[:, 0:W], mybir.AxisListType.X,
                        ALU.add)
                    nc.vector.tensor_reduce(
                        rv[:, 2, j:j + 1], s[:, PIX - W:PIX],
                        mybir.AxisListType.X, ALU.add)
                    nc.vector.tensor_reduce(
                        rv[:, 3, j:j + 1], s3[:, :, 0:1],
                        mybir.AxisListType.XY, ALU.add)
                    nc.vector.tensor_reduce(
                        rv[:, 4, j:j + 1], s3[:, :, W - 1:W],
                        mybir.AxisListType.XY, ALU.add)
                    corners = s3[:, 0:28:27, 0:28:27]   # [128, 2, 2]
                    dstc = rv[:, 5:9, j:j + 1].rearrange(
                        "c (x y) o -> c x (y o)", x=2)
                    nc.vector.tensor_copy(dstc, corners)

    # ---- conv windows ----
    gt_tiles = {}

    def emit_conv(j):
        for wi in range(NW):
            for dc in range(NDC):
                h0 = 1 + wi * WROWS
                base = (h0 - 1) * W
                p = ps_conv.tile([128, WN], F32, tag="pconv", name="pconv")
                p3 = _r3(p[:], W)
                nc.tensor.matmul(p[:], identbf[:], gbias[dc][:, base:base + WN],
                                 start=True, stop=False)
                for conv, tn in ((0, "in"), (1, "hid")):
                    for t, (kh, kw) in enumerate(TAPS):
                        dh, dwid = kh - 1, kw - 1
                        for cc in range(NCC):
                            rhs = _r3(xt[(j, tn, cc)][:], PAD)[
                                :, h0 + dh:h0 + dh + WROWS,
                                1 + dwid:1 + dwid + W]
                            last = (conv == 1 and t == 8 and cc == NCC - 1)
                            nc.tensor.matmul(
                                p3, wblk(conv, t, cc)[:, dc * 128:(dc + 1) * 128],
                                rhs, start=False, stop=last)
                gt = gtp.tile([128, WN], F32, tag="gt", name="gt")
                nc.scalar.activation(gt[:], p[:], AF.Tanh)
                gt_tiles[(j, wi, dc)] = gt

    # ---- gap combine + AllGather ----
    def emit_combine():
        gap_ps = ps_gap.tile([8, D], F32, tag="gapI")
        for cc in range(NCC):
            rv = rawI[cc][:].rearrange("c (g b) -> c g b", b=BL)
            for g in range(9):
                nc.tensor.matmul(gap_ps[:], rv[:, g], ablk(cc, g),
                                 start=(cc == 0 and g == 0),
                                 stop=(cc == NCC - 1 and g == 8))
        gapI_sb = const.tile([8, D], F32, tag="gapI_sb")
        nc.vector.tensor_copy(gapI_sb[:], gap_ps[:])
        nc.scalar.dma_start(dram["cc_in"][0:8, :], gapI_sb[:, 0:128])
        nc.scalar.dma_start(dram["cc_in"][8:16, :], gapI_sb[:, 128:256])
        for cc in range(NCC):
            pt = ps_tr.tile([128, 128], F32, tag="ptr", name="pt_gapH")
            pt = pt[0:8, :]
            nc.tensor.transpose(pt, rawH[cc][:], ident[:])
            hs = const.tile([8, 128], F32, tag=f"gapH_sb{cc}",
                            name=f"gapHsb{cc}")
            nc.vector.tensor_copy(hs[:], pt)
            nc.scalar.dma_start(dram["cc_in"][16 + 8 * cc:24 + 8 * cc, :],
                                hs[:])
        nc.gpsimd.collective_compute(
            "AllGather", ALU.bypass, replica_groups=[list(range(N_CORES))],
            ins=[dram["cc_in"][:]], outs=[dram["cc_out"][:]])

    # ---- gather + gate tables ----
    gates = {}

    def emit_gates():
        sel = [const.tile([128, 224], F32, tag=f"sel{g4}", name=f"sel{g4}")
               for g4 in range(4)]
        for g4 in range(4):
            for hf in range(2):
                gtile = gath.tile([112, 128], F32, tag="gath", name="gath")
                nc.gpsimd.indirect_dma_start(
                    out=gtile[:], out_offset=None, in_=dram["cc_out"][:],
                    in_offset=bass.IndirectOffsetOnAxis(
                        ap=idx_sb[g4][hf][:, :1], axis=0))
                pt = ps_tr.tile([128, 128], F32, tag="ptr", name="pt_gath")
                nc.tensor.transpose(pt[:, 0:112], gtile[:],
                                    ident[0:112, 0:112])
                nc.vector.tensor_copy(sel[g4][:, hf * 112:(hf + 1) * 112],
                                      pt[:, 0:112])
        for gate, wx, wh, bi in (("i", "wxi", "whi", "inputBias"),
                                 ("f", "wxf", "whf", "forgetBias"),
                                 ("o", "wxo", "who", "outputBias")):
            per_dc = []
            for dc in range(NDC):
                t1 = gtmp.tile([128, 224], F32, tag="gm1", name="gm1")
                nc.vector.tensor_scalar_mul(t1[:], sel[dc][:],
                                            vecs[wx][:, dc:dc + 1])
                t2 = gtmp.tile([128, 224], F32, tag="gm2", name="gm2")
                nc.vector.tensor_scalar_mul(t2[:], sel[2 + dc][:],
                                            vecs[wh][:, dc:dc + 1])
                nc.vector.tensor_tensor(out=t1[:], in0=t1[:], in1=t2[:],
                                        op=ALU.add)
                gt = const.tile([128, 224], F32, tag=f"gate_{gate}{dc}",
                                name=f"gate_{gate}{dc}")
                nc.scalar.activation(gt[:], t1[:], AF.Sigmoid,
                                     bias=vecs[bi][:, dc:dc + 1])
                per_dc.append(gt)
            gates[gate] = per_dc

    # ---- elementwise + store ----
    def emit_ew(j):
        stT = [outb.tile([128, PIX], F32, tag="stT", name=f"stT{j}_{dc}")
               for dc in range(NDC)]
        hidT = [outb.tile([128, PIX], F32, tag="hidT", name=f"hidT{j}_{dc}")
                for dc in range(NDC)]
        for wi in range(NW):
            for dc in range(NDC):
                h0 = 1 + wi * WROWS
                base = (h0 - 1) * W
                t0 = j * H + (h0 - 1)

                def gw(gate):
                    return gates[gate][dc][:, t0:t0 + WROWS].to_broadcast(
                        [128, WROWS, W])

                gt = gt_tiles.pop((j, wi, dc))
                sp3 = _r3(xsp_tiles[j][dc][:, base:base + WN], W)
                g3 = _r3(gt[:], W)
                st3 = _r3(stT[dc][:, base:base + WN], W)
                hd3 = _r3(hidT[dc][:, base:base + WN], W)
                s1 = ew.tile([128, WN], F32, tag="s1", name="s1")
                nc.gpsimd.tensor_tensor(out=_r3(s1[:], W), in0=sp3,
                                        in1=gw("f"), op=ALU.mult)
                s2 = ew.tile([128, WN], F32, tag="s2", name="s2")
                nc.gpsimd.tensor_tensor(out=_r3(s2[:], W), in0=g3,
                                        in1=gw("i"), op=ALU.mult)
                nc.vector.tensor_tensor(out=st3, in0=_r3(s1[:], W),
                                        in1=_r3(s2[:], W), op=ALU.add)
                th = ew.tile([128, WN], F32, tag="th", name="th")
                nc.scalar.activation(th[:], stT[dc][:, base:base + WN],
                                     AF.Tanh)
                nc.gpsimd.tensor_tensor(out=hd3, in0=_r3(th[:], W),
                                        in1=gw("o"), op=ALU.mult)
        for dname, buf in (("state", stT), ("hidden", hidT)):
            for dc in range(NDC):
                nc.scalar.dma_start(dram[dname][j, dc], buf[dc][:])

    # ================= schedule =================
    for j in range(BL):
        emit_load(j)
    for j in range(BL):
        emit_load_state(j)
    for j in range(3):
        emit_stage(j)
    emit_conv(0)
    emit_conv(1)
    for j in range(3, BL):
        emit_stage(j)
    emit_combine()
    emit_conv(2)
    emit_gates()
    emit_conv(3)
    emit_ew(0)
    for j in range(4, BL):
        emit_conv(j)
        emit_ew(j - 3)
    for j in range(BL - 3, BL):
        emit_ew(j)

    ctx.close()


# ---------------------------------------------------------------------------
_NC_CACHE = None


def _get_nc():
    global _NC_CACHE
    if _NC_CACHE is None:
        nc = build_nc()
        _split_excess_sem_waits(nc)
        _NC_CACHE = nc
    return _NC_CACHE


def _gate_idx(core):
    idx = np.empty((4, 224, 1), np.int32)
    for j in range(BL):
        for hh in range(H):
            t = j * H + hh
            sel_b = (H * (BL * core + j) + hh) % B
            cp, bp = sel_b // BL, sel_b % BL
            for g in range(4):
                idx[g, t, 0] = cp * 32 + g * 8 + bp
    return idx


def _make_in_maps(inputs):
    f32 = np.float32
    bf16 = ml_dtypes.bfloat16

    w1 = np.ascontiguousarray(inputs["wconvInput"], dtype=f32)  # [3,3,CIN,D]
    w2 = np.ascontiguousarray(inputs["wconvHidden"], dtype=f32)
    # wconv_bf[conv, t, cc, 128, D]
    wconv = np.empty((2, 9, NCC, 128, D), dtype=bf16)
    for conv, w in ((0, w1), (1, w2)):
        for t, (kh, kw) in enumerate(TAPS):
            for cc in range(NCC):
                wconv[conv, t, cc] = w[kh, kw, cc * 128:(cc + 1) * 128, :]

    # A-fold for gapI: 784*gapI = sum_g raw_g^T @ A_g  (group order
    # S, Rf, Rl, Cf, Cl, K00, K0L, KL0, KLL; edge groups negated)
    wt = w1.reshape(9, CIN, D)
    A = np.empty((9, CIN, D), f32)
    A[0] = wt.sum(0)
    A[1] = -(wt[6] + wt[7] + wt[8])
    A[2] = -(wt[0] + wt[1] + wt[2])
    A[3] = -(wt[2] + wt[5] + wt[8])
    A[4] = -(wt[0] + wt[3] + wt[6])
    A[5], A[6], A[7], A[8] = wt[8], wt[6], wt[2], wt[0]
    afold = np.empty((NCC, 9, 128, D), dtype=bf16)
    for cc in range(NCC):
        afold[cc] = A[:, cc * 128:(cc + 1) * 128, :]

    gb = np.ascontiguousarray(inputs["gateBias"], dtype=f32).reshape(PIX, D)
    gbias_t = np.ascontiguousarray(gb.T.reshape(NDC, 128, PIX)).astype(bf16)

    shared = {
        "wconv_bf": wconv,
        "a_fold": afold,
        "gbias_t": gbias_t,
        "identity": np.eye(128, dtype=f32),
        "identity_bf": np.eye(128, dtype=f32).astype(bf16),
    }
    for nm in ("wxi", "whi", "inputBias", "wxf", "whf", "forgetBias",
               "wxo", "who", "outputBias"):
        v = np.ascontiguousarray(inputs[nm], dtype=f32).reshape(D)
        if nm.startswith("wx") or nm.startswith("wh"):
            v = v / PIX
        shared[nm] = np.ascontiguousarray(v.reshape(NDC, 128).T)  # [128, NDC]

    def chan_major(x, dtype):
        # [B, PIX, C] -> [B, NCC, 128, PIX]
        xt = np.ascontiguousarray(x.reshape(B, PIX, CIN).transpose(0, 2, 1))
        return xt.reshape(B, NCC, 128, PIX).astype(dtype)

    xin = chan_major(np.asarray(inputs["inputs"], dtype=f32), bf16)
    xhp = chan_major(np.asarray(inputs["hidden_prev"], dtype=f32), bf16)
    xsp = chan_major(np.asarray(inputs["state_prev"], dtype=f32), f32)

    in_maps = []
    for k in range(N_CORES):
        sl = slice(k * BL, (k + 1) * BL)
        m = dict(shared)
        m["inputs_t"] = xin[sl]
        m["hidden_t"] = xhp[sl]
        m["state_t"] = xsp[sl]
        m["gate_idx"] = _gate_idx(k)
        in_maps.append(m)
    return in_maps


def kernel(**inputs):
    nc = _get_nc()
    in_maps = _make_in_maps(inputs)
    res = run_bass_kernel_spmd(nc, in_maps, core_ids=list(range(N_CORES)))

    def unshard(name):
        # per-core outputs are [BL, NDC, 128, PIX] (channel-major)
        full = np.concatenate([res.results[k][name] for k in range(N_CORES)],
                              axis=0)
        return np.ascontiguousarray(full.transpose(0, 3, 1, 2)).reshape(
            B, H, W, D)

    return unshard("hidden"), unshard("state")


# revision 20
# speedup vs baseline: 1.5222x; 1.0901x over previous
"""Trainium2 Bass kernel for nn_DeformableConvLSTMCell_33895881900284.

Full (unsharded) inputs in, full outputs out. Data-parallel over batch across
8 NeuronCores (8 batches per core), conv weights / gate params replicated.

Math per the reference:
  outI  = conv3x3_same(inputs, wconvInput)
  g     = tanh(outI + conv3x3_same(hidden_prev, wconvHidden) + gateBias)
  gapI  = mean_hw(outI);  gapH = mean_hw(hidden_prev)          # [B, D]
  i/f/o = sigmoid(wx*gapI + wh*gapH + bias)                    # [B, D]
  tiled gate: value used at (b, h, w, c) is gate[(28*b + h) % 64, c]
  state  = f*state_prev + i*g;  hidden = o*tanh(state)

v2 design (vs the f32r baseline):
  * Host pre-transposes inputs/hidden/state to channel-major [BL, cc, 128,
    784] (layout-only, like the baseline's host-side output untranspose), so
    the kernel does no PE transposes and no DVE layout copies.
  * The whole conv path runs in bf16: weights/inputs/gateBias are cast on
    the host, conv = 36 shifted bf16 matmuls + a bias identity-matmul per
    392-pixel window accumulating in fp32 PSUM. bf16 enables the PE's Fast
    Weight Load path (fp32r disables it), roughly halving per-matmul cost.
  * gapI never touches the conv output: by linearity 784*gapI is a
    combination of 9 masked pixel sums of the raw input with host-folded
    A-matrices (sums of conv taps). The masked sums (full/edge/corner) are
    free-dim DVE reductions over the staged channel-major tiles, so the
    AllGather of gate drivers fires ~45us in, long before the first
    elementwise consumer.
  * The (28*b+h)%64 gate scrambling makes gates cross-batch: cores AllGather
    their local GAP columns and a per-core index-array input drives an
    indirect-DMA gather of exactly the gate rows this core needs (the SPMD
    program stays identical across cores; only input data differs).
"""
import numpy as np
import ml_dtypes

import bass_rust
import concourse.bass as bass
import concourse.mybir as mybir
import concourse.tile as tile
from concourse.bass_utils import run_bass_kernel_spmd

F32 = mybir.dt.float32
BF16 = mybir.dt.bfloat16
I32 = mybir.dt.int32
AF = mybir.ActivationFunctionType
ALU = mybir.AluOpType

N_CORES = 8
B, H, W, CIN, D = 64, 28, 28, 256, 256
BL = B // N_CORES          # local batches per core
PIX = H * W                # 784
PAD = 30                   # padded row/col length
XTLEN = PAD * PAD          # 900
NW = 2                     # windows per batch
WROWS = H // NW            # 14
WN = WROWS * W             # 392
NCC = CIN // 128           # 2 input-channel chunks
NDC = D // 128             # 2 output-channel chunks

# tap order t = 3*kh + kw ; dh = kh-1, dw = kw-1
TAPS = [(kh, kw) for kh in range(3) for kw in range(3)]

# ---------------------------------------------------------------------------
# walrus fixup: split semaphore waits that exceed the per-instruction budget
MAX_WAITS = 1


def _split_excess_sem_waits(nc):
    counter = [0]
    for fn in nc.m.functions:
        for bb in fn.blocks:
            insts = bb.instructions
            i = 0
            while i < len(insts):
                inst = insts[i]
                si = inst.sync_info
                if si is not None and si.on_wait and len(si.on_wait) > MAX_WAITS:
                    waits = list(si.on_wait)
                    excess = waits[:-MAX_WAITS]
                    keep = waits[-MAX_WAITS:]
                    new_insts = []
                    for j in range(0, len(excess), MAX_WAITS):
                        chunk = excess[j:j + MAX_WAITS]
                        noop = mybir.InstNoOp(
                            name=f"I-waitsplit-{counter[0]}", ins=[], outs=[])
                        counter[0] += 1
                        noop.engine = inst.engine
                        noop.sync_info = bass_rust.SyncInfo(
                            on_wait=chunk, on_update=[])
                        nc.register_instruction(noop)
                        new_insts.append(noop)
                    inst.sync_info = bass_rust.SyncInfo(
                        on_wait=keep, on_update=list(si.on_update))
                    insts[i:i] = new_insts
                    i += len(new_insts)
                i += 1
    return nc


# ---------------------------------------------------------------------------
def _r3(ap, w):
    """view flat free dim as (rows, w)"""
    return ap.rearrange("c (r w) -> c r w", w=w)


def build_nc():
    nc = bass.Bass("TRN2", target_bir_lowering=False, debug=False,
                   num_devices=N_CORES)

    dram = {}
    dram["xin"] = nc.dram_tensor("inputs_t", [BL, NCC, 128, PIX], BF16,
                                 kind="ExternalInput").ap()
    dram["xhid"] = nc.dram_tensor("hidden_t", [BL, NCC, 128, PIX], BF16,
                                  kind="ExternalInput").ap()
    dram["xsp"] = nc.dram_tensor("state_t", [BL, NCC, 128, PIX], F32,
                                 kind="ExternalInput").ap()
    dram["w1"] = nc.dram_tensor("w1_bf", [9, NCC, 128, D], BF16,
                                kind="ExternalInput").ap()
    dram["w2"] = nc.dram_tensor("w2_bf", [9, NCC, 128, D], BF16,
                                kind="ExternalInput").ap()
    dram["afold"] = nc.dram_tensor("a_fold", [NCC, 9, 128, D], BF16,
                                   kind="ExternalInput").ap()
    dram["gb"] = nc.dram_tensor("gbias_t", [NDC, 128, PIX], F32,
                                kind="ExternalInput").ap()
    dram["vec"] = {}
    for nm in ("wxi", "whi", "inputBias", "wxf", "whf", "forgetBias",
               "wxo", "who", "outputBias"):
        dram["vec"][nm] = nc.dram_tensor(nm, [128, NDC], F32,
                                         kind="ExternalInput").ap()
    dram["ident"] = nc.dram_tensor("identity", [128, 128], F32,
                                   kind="ExternalInput").ap()
    dram["idx"] = nc.dram_tensor("gate_idx", [4, 224, 1], I32,
                                 kind="ExternalInput").ap()
    dram["hidden"] = nc.dram_tensor("hidden", [BL, NDC, 128, PIX], F32,
                                    kind="ExternalOutput").ap()
    dram["state"] = nc.dram_tensor("state", [BL, NDC, 128, PIX], F32,
                                   kind="ExternalOutput").ap()
    dram["cc_in"] = nc.dram_tensor("cc_in", [32, 128], F32, kind="Internal").ap()
    dram["cc_out"] = nc.dram_tensor("cc_out", [N_CORES * 32, 128], F32,
                                    kind="Internal", addr_space="Shared").ap()

    ctx_mgr = nc.allow_low_precision("bf16 conv path")
    ctx_mgr.__enter__()
    with tile.TileContext(nc) as tc:
        _build_body(nc, tc, dram)
    ctx_mgr.__exit__(None, None, None)
    return nc


def _build_body(nc, tc, dram):
    from contextlib import ExitStack
    ctx = ExitStack()
    pool = lambda **kw: ctx.enter_context(tc.tile_pool(**kw))

    const = pool(name="const", bufs=1)
    stg = pool(name="stg", bufs=4)         # [128, 784] bf16 staging
    xspp = pool(name="xspp", bufs=6)       # [128, 784] f32 state tiles
    gtp = pool(name="gtp", bufs=14)        # [128, 392] f32 tanh-conv tiles
    outb = pool(name="outb", bufs=4)       # [128, 784] f32 out tiles (per tag)
    ew = pool(name="ew", bufs=3)           # [128, 392] f32 scratch (per tag)
    gtmp = pool(name="gtmp", bufs=2)
    gath = pool(name="gath", bufs=2)
    ps_conv = pool(name="ps_conv", bufs=6, space="PSUM")
    ps_gap = pool(name="ps_gap", bufs=1, space="PSUM")
    ps_tr = pool(name="ps_tr", bufs=1, space="PSUM")

    # ---- constants; weights split across both DMA rings so conv(0) can
    # start ~8us in: w1 leads the sync ring, w2 follows gbias on scalar ----
    wc = [const.tile([128, 9 * NCC * D], BF16, tag=f"wc{conv}",
                     name=f"wc{conv}") for conv in range(2)]

    def wblk(conv, t, cc):
        off = (t * NCC + cc) * D
        return wc[conv][:, off:off + D]

    # dram [9,NCC,128,D] -> SBUF [128, (9,NCC,D)] in one strided DMA
    nc.sync.dma_start(
        wc[0][:].rearrange("p (t c n) -> p t c n", t=9, c=NCC),
        dram["w1"][:].rearrange("t c p n -> p t c n"))

    gbias = [const.tile([128, PIX], F32, tag=f"gbias{dc}", name=f"gbias{dc}")
             for dc in range(NDC)]
    for dc in range(NDC):
        nc.scalar.dma_start(gbias[dc][:], dram["gb"][dc])

    nc.scalar.dma_start(
        wc[1][:].rearrange("p (t c n) -> p t c n", t=9, c=NCC),
        dram["w2"][:].rearrange("t c p n -> p t c n"))

    afold = const.tile([128, NCC * 9 * D], BF16, tag="afold")

    def ablk(cc, g):
        off = (cc * 9 + g) * D
        return afold[:, off:off + D]

    nc.scalar.dma_start(
        afold[:].rearrange("p (c g n) -> p c g n", c=NCC, g=9),
        dram["afold"][:].rearrange("c g p n -> p c g n"))

    idx_sb = []
    for g4 in range(4):
        halves = []
        for hf in range(2):
            t = const.tile([112, 1], I32, tag=f"idx{g4}_{hf}")
            nc.scalar.dma_start(t[:], dram["idx"][g4, hf * 112:(hf + 1) * 112, :])
            halves.append(t)
        idx_sb.append(halves)

    vecs = {}
    for nm in dram["vec"]:
        t = const.tile([128, NDC], F32, tag=f"vec_{nm}")
        nc.scalar.dma_start(t[:], dram["vec"][nm][:])
        vecs[nm] = t

    ident = const.tile([128, 128], F32, tag="ident")
    nc.scalar.dma_start(ident[:], dram["ident"][:])

    # ---- padded input tiles: one fixed slot per (batch, tensor, cc) ----
    # memset zeroes the whole tile once; only the 28x28 interior is
    # rewritten per run, so the pad border stays zero.
    xt = {}       # xt[(j, tensor, cc)] -> [128, 900] bf16
    for j in range(BL):
        for tn in ("in", "hid"):
            for cc in range(NCC):
                t = const.tile([128, XTLEN], BF16, tag=f"x{tn}{j}_{cc}",
                               name=f"x{tn}{j}_{cc}")
                nc.gpsimd.memset(t[:], 0.0)
                xt[(j, tn, cc)] = t

    # masked-sum accumulators (columns written per batch)
    rawI = [const.tile([128, 9 * BL], BF16, tag=f"rawI{cc}", name=f"rawI{cc}")
            for cc in range(NCC)]
    rawH = [const.tile([128, BL], F32, tag=f"rawH{cc}", name=f"rawH{cc}")
            for cc in range(NCC)]

    # ---- per-batch input loading (sync/SP DMA ring) ----
    stg_tiles = {}

    def emit_load(j):
        for tn, dsrc in (("in", dram["xin"]), ("hid", dram["xhid"])):
            for cc in range(NCC):
                s = stg.tile([128, PIX], BF16, tag="stg",
                             name=f"stg{tn}{j}_{cc}")
                nc.sync.dma_start(s[:], dsrc[j, cc])
                stg_tiles[(j, tn, cc)] = s

    xsp_tiles = {}

    def emit_load_state(j):
        ts = []
        for cc in range(NCC):
            s = xspp.tile([128, PIX], F32, tag="xsp", name=f"xsp{j}_{cc}")
            nc.sync.dma_start(s[:], dram["xsp"][j, cc])
            ts.append(s)
        xsp_tiles[j] = ts

    # ---- stage: pad-copy (scalar) + masked-sum reductions (vector) ----
    def emit_stage(j):
        for tn in ("in", "hid"):
            for cc in range(NCC):
                s = stg_tiles.pop((j, tn, cc))
                s3 = _r3(s[:], W)                       # [128, 28, 28]
                dst = _r3(xt[(j, tn, cc)][:], PAD)[:, 1:29, 1:29]
                nc.scalar.copy(dst, s3)
                if tn == "hid":
                    # gapH: full pixel sum only
                    nc.vector.tensor_reduce(
                        rawH[cc][:, j:j + 1], s[:], mybir.AxisListType.X,
                        ALU.add)
                else:
                    rv = rawI[cc][:].rearrange("c (g b) -> c g b", b=BL)
                    # group order: S, Rf(row0), Rl(row27), Cf(col0),
                    # Cl(col27), K00, K0L, KL0, KLL
                    nc.vector.tensor_reduce(
                        rv[:, 0, j:j + 1], s[:], mybir.AxisListType.X, ALU.add)
                    nc.vector.tensor_reduce(
                        rv[:, 1, j:j + 1], s[:, 0:W], mybir.AxisListType.X,
                        ALU.add)
                    nc.vector.tensor_reduce(
                        rv[:, 2, j:j + 1], s[:, PIX - W:PIX],
                        mybir.AxisListType.X, ALU.add)
                    nc.vector.tensor_reduce(
                        rv[:, 3, j:j + 1], s3[:, :, 0:1],
                        mybir.AxisListType.XY, ALU.add)
                    nc.vector.tensor_reduce(
                        rv[:, 4, j:j + 1], s3[:, :, W - 1:W],
                        mybir.AxisListType.XY, ALU.add)
                    corners = s3[:, 0:28:27, 0:28:27]   # [128, 2, 2]
                    dstc = rv[:, 5:9, j:j + 1].rearrange(
                        "c (x y) o -> c x (y o)", x=2)
                    nc.vector.tensor_copy(dstc, corners)

    # ---- conv windows ----
    gt_tiles = {}

    def emit_conv(j):
        for wi in range(NW):
            for dc in range(NDC):
                h0 = 1 + wi * WROWS
                base = (h0 - 1) * W
                p = ps_conv.tile([128, WN], F32, tag="pconv", name="pconv")
                p3 = _r3(p[:], W)
                first = True
                for conv, tn in ((0, "in"), (1, "hid")):
                    for t, (kh, kw) in enumerate(TAPS):
                        dh, dwid = kh - 1, kw - 1
                        for cc in range(NCC):
                            rhs = _r3(xt[(j, tn, cc)][:], PAD)[
                                :, h0 + dh:h0 + dh + WROWS,
                                1 + dwid:1 + dwid + W]
                            last = (conv == 1 and t == 8 and cc == NCC - 1)
                            nc.tensor.matmul(
                                p3, wblk(conv, t, cc)[:, dc * 128:(dc + 1) * 128],
                                rhs, start=first, stop=last)
                            first = False
                nc.vector.tensor_tensor(out=p[:], in0=p[:],
                                        in1=gbias[dc][:, base:base + WN],
                                        op=ALU.add)
                gt = gtp.tile([128, WN], F32, tag="gt", name="gt")
                nc.scalar.activation(gt[:], p[:], AF.Tanh)
                gt_tiles[(j, wi, dc)] = gt

    # ---- gap combine + AllGather ----
    def emit_combine():
        gap_ps = ps_gap.tile([8, D], F32, tag="gapI")
        for cc in range(NCC):
            rv = rawI[cc][:].rearrange("c (g b) -> c g b", b=BL)
            for g in range(9):
                nc.tensor.matmul(gap_ps[:], rv[:, g], ablk(cc, g),
                                 start=(cc == 0 and g == 0),
                                 stop=(cc == NCC - 1 and g == 8))
        gapI_sb = const.tile([8, D], F32, tag="gapI_sb")
        nc.vector.tensor_copy(gapI_sb[:], gap_ps[:])
        nc.scalar.dma_start(dram["cc_in"][0:8, :], gapI_sb[:, 0:128])
        nc.scalar.dma_start(dram["cc_in"][8:16, :], gapI_sb[:, 128:256])
        for cc in range(NCC):
            pt = ps_tr.tile([128, 128], F32, tag="ptr", name="pt_gapH")
            pt = pt[0:8, :]
            nc.tensor.transpose(pt, rawH[cc][:], ident[:])
            hs = const.tile([8, 128], F32, tag=f"gapH_sb{cc}",
                            name=f"gapHsb{cc}")
            nc.vector.tensor_copy(hs[:], pt)
            nc.scalar.dma_start(dram["cc_in"][16 + 8 * cc:24 + 8 * cc, :],
                                hs[:])
        nc.gpsimd.collective_compute(
            "AllGather", ALU.bypass, replica_groups=[list(range(N_CORES))],
            ins=[dram["cc_in"][:]], outs=[dram["cc_out"][:]])

    # ---- gather + gate tables ----
    gates = {}

    def emit_gates():
        sel = [const.tile([128, 224], F32, tag=f"sel{g4}", name=f"sel{g4}")
               for g4 in range(4)]
        for g4 in range(4):
            for hf in range(2):
                gtile = gath.tile([112, 128], F32, tag="gath", name="gath")
                nc.gpsimd.indirect_dma_start(
                    out=gtile[:], out_offset=None, in_=dram["cc_out"][:],
                    in_offset=bass.IndirectOffsetOnAxis(
                        ap=idx_sb[g4][hf][:, :1], axis=0))
                pt = ps_tr.tile([128, 128], F32, tag="ptr", name="pt_gath")
                nc.tensor.transpose(pt[:, 0:112], gtile[:],
                                    ident[0:112, 0:112])
                nc.vector.tensor_copy(sel[g4][:, hf * 112:(hf + 1) * 112],
                                      pt[:, 0:112])
        for gate, wx, wh, bi in (("i", "wxi", "whi", "inputBias"),
                                 ("f", "wxf", "whf", "forgetBias"),
                                 ("o", "wxo", "who", "outputBias")):
            per_dc = []
            for dc in range(NDC):
                t1 = gtmp.tile([128, 224], F32, tag="gm1", name="gm1")
                nc.vector.tensor_scalar_mul(t1[:], sel[dc][:],
                                            vecs[wx][:, dc:dc + 1])
                t2 = gtmp.tile([128, 224], F32, tag="gm2", name="gm2")
                nc.vector.tensor_scalar_mul(t2[:], sel[2 + dc][:],
                                            vecs[wh][:, dc:dc + 1])
                nc.vector.tensor_tensor(out=t1[:], in0=t1[:], in1=t2[:],
                                        op=ALU.add)
                gt = const.tile([128, 224], F32, tag=f"gate_{gate}{dc}",
                                name=f"gate_{gate}{dc}")
                nc.scalar.activation(gt[:], t1[:], AF.Sigmoid,
                                     bias=vecs[bi][:, dc:dc + 1])
                per_dc.append(gt)
            gates[gate] = per_dc

    # ---- elementwise + store ----
    def emit_ew(j):
        stT = [outb.tile([128, PIX], F32, tag="stT", name=f"stT{j}_{dc}")
               for dc in range(NDC)]
        hidT = [outb.tile([128, PIX], F32, tag="hidT", name=f"hidT{j}_{dc}")
                for dc in range(NDC)]
        for wi in range(NW):
            for dc in range(NDC):
                h0 = 1 + wi * WROWS
                base = (h0 - 1) * W
                t0 = j * H + (h0 - 1)

                def gw(gate):
                    return gates[gate][dc][:, t0:t0 + WROWS].to_broadcast(
                        [128, WROWS, W])

                gt = gt_tiles.pop((j, wi, dc))
                sp3 = _r3(xsp_tiles[j][dc][:, base:base + WN], W)
                g3 = _r3(gt[:], W)
                st3 = _r3(stT[dc][:, base:base + WN], W)
                hd3 = _r3(hidT[dc][:, base:base + WN], W)
                s1 = ew.tile([128, WN], F32, tag="s1", name="s1")
                nc.gpsimd.tensor_tensor(out=_r3(s1[:], W), in0=sp3,
                                        in1=gw("f"), op=ALU.mult)
                s2 = ew.tile([128, WN], F32, tag="s2", name="s2")
                nc.gpsimd.tensor_tensor(out=_r3(s2[:], W), in0=g3,
                                        in1=gw("i"), op=ALU.mult)
                nc.vector.tensor_tensor(out=st3, in0=_r3(s1[:], W),
                                        in1=_r3(s2[:], W), op=ALU.add)
                th = ew.tile([128, WN], F32, tag="th", name="th")
                nc.scalar.activation(th[:], stT[dc][:, base:base + WN],
                                     AF.Tanh)
                nc.gpsimd.tensor_tensor(out=hd3, in0=_r3(th[:], W),
                                        in1=gw("o"), op=ALU.mult)
        for dname, buf in (("state", stT), ("hidden", hidT)):
            for dc in range(NDC):
                nc.scalar.dma_start(dram[dname][j, dc], buf[dc][:])

    # ================= schedule =================
    for j in range(BL):
        emit_load(j)
    for j in range(BL):
        emit_load_state(j)
    for j in range(BL):
        emit_stage(j)
    emit_conv(0)
    emit_combine()
    emit_conv(1)
    emit_conv(2)
    emit_gates()
    emit_conv(3)
    emit_ew(0)
    for j in range(4, BL):
        emit_conv(j)
        emit_ew(j - 3)
    for j in range(BL - 3, BL):
        emit_ew(j)

    ctx.close()


# ---------------------------------------------------------------------------
_NC_CACHE = None


def _get_nc():
    global _NC_CACHE
    if _NC_CACHE is None:
        nc = build_nc()
        _split_excess_sem_waits(nc)
        _NC_CACHE = nc
    return _NC_CACHE


def _gate_idx(core):
    idx = np.empty((4, 224, 1), np.int32)
    for j in range(BL):
        for hh in range(H):
            t = j * H + hh
            sel_b = (H * (BL * core + j) + hh) % B
            cp, bp = sel_b // BL, sel_b % BL
            for g in range(4):
                idx[g, t, 0] = cp * 32 + g * 8 + bp
    return idx


def _make_in_maps(inputs):
    f32 = np.float32
    bf16 = ml_dtypes.bfloat16

    w1 = np.ascontiguousarray(inputs["wconvInput"], dtype=f32)  # [3,3,CIN,D]
    w2 = np.ascontiguousarray(inputs["wconvHidden"], dtype=f32)
    # w{1,2}_bf[t, cc, 128, D]
    w1b = np.empty((9, NCC, 128, D), dtype=bf16)
    w2b = np.empty((9, NCC, 128, D), dtype=bf16)
    for wb, w in ((w1b, w1), (w2b, w2)):
        for t, (kh, kw) in enumerate(TAPS):
            for cc in range(NCC):
                wb[t, cc] = w[kh, kw, cc * 128:(cc + 1) * 128, :]

    # A-fold for gapI: 784*gapI = sum_g raw_g^T @ A_g  (group order
    # S, Rf, Rl, Cf, Cl, K00, K0L, KL0, KLL; edge groups negated)
    wt = w1.reshape(9, CIN, D)
    A = np.empty((9, CIN, D), f32)
    A[0] = wt.sum(0)
    A[1] = -(wt[6] + wt[7] + wt[8])
    A[2] = -(wt[0] + wt[1] + wt[2])
    A[3] = -(wt[2] + wt[5] + wt[8])
    A[4] = -(wt[0] + wt[3] + wt[6])
    A[5], A[6], A[7], A[8] = wt[8], wt[6], wt[2], wt[0]
    afold = np.empty((NCC, 9, 128, D), dtype=bf16)
    for cc in range(NCC):
        afold[cc] = A[:, cc * 128:(cc + 1) * 128, :]

    gb = np.ascontiguousarray(inputs["gateBias"], dtype=f32).reshape(PIX, D)
    gbias_t = np.ascontiguousarray(gb.T.reshape(NDC, 128, PIX))

    shared = {
        "w1_bf": w1b,
        "w2_bf": w2b,
        "a_fold": afold,
        "gbias_t": gbias_t,
        "identity": np.eye(128, dtype=f32),
    }
    for nm in ("wxi", "whi", "inputBias", "wxf", "whf", "forgetBias",
               "wxo", "who", "outputBias"):
        v = np.ascontiguousarray(inputs[nm], dtype=f32).reshape(D)
        if nm.startswith("wx") or nm.startswith("wh"):
            v = v / PIX
        shared[nm] = np.ascontiguousarray(v.reshape(NDC, 128).T)  # [128, NDC]

    def chan_major(x, dtype):
        # [B, PIX, C] -> [B, NCC, 128, PIX]
        xt = np.ascontiguousarray(x.reshape(B, PIX, CIN).transpose(0, 2, 1))
        return xt.reshape(B, NCC, 128, PIX).astype(dtype)

    xin = chan_major(np.asarray(inputs["inputs"], dtype=f32), bf16)
    xhp = chan_major(np.asarray(inputs["hidden_prev"], dtype=f32), bf16)
    xsp = chan_major(np.asarray(inputs["state_prev"], dtype=f32), f32)

    in_maps = []
    for k in range(N_CORES):
        sl = slice(k * BL, (k + 1) * BL)
        m = dict(shared)
        m["inputs_t"] = xin[sl]
        m["hidden_t"] = xhp[sl]
        m["state_t"] = xsp[sl]
        m["gate_idx"] = _gate_idx(k)
        in_maps.append(m)
    return in_maps


def kernel(**inputs):
    nc = _get_nc()
    in_maps = _make_in_maps(inputs)
    res = run_bass_kernel_spmd(nc, in_maps, core_ids=list(range(N_CORES)))

    def unshard(name):
        # per-core outputs are [BL, NDC, 128, PIX] (channel-major)
        full = np.concatenate([res.results[k][name] for k in range(N_CORES)],
                              axis=0)
        return np.ascontiguousarray(full.transpose(0, 3, 1, 2)).reshape(
            B, H, W, D)

    return unshard("hidden"), unshard("state")


# revision 26
# speedup vs baseline: 1.6327x; 1.0726x over previous
"""Trainium2 Bass kernel for nn_DeformableConvLSTMCell_33895881900284.

Full (unsharded) inputs in, full outputs out. Data-parallel over batch across
8 NeuronCores (8 batches per core), conv weights / gate params replicated.

Math per the reference:
  outI  = conv3x3_same(inputs, wconvInput)
  g     = tanh(outI + conv3x3_same(hidden_prev, wconvHidden) + gateBias)
  gapI  = mean_hw(outI);  gapH = mean_hw(hidden_prev)          # [B, D]
  i/f/o = sigmoid(wx*gapI + wh*gapH + bias)                    # [B, D]
  tiled gate: value used at (b, h, w, c) is gate[(28*b + h) % 64, c]
  state  = f*state_prev + i*g;  hidden = o*tanh(state)

v2 design (vs the f32r baseline):
  * Host pre-transposes inputs/hidden/state to channel-major [BL, cc, 128,
    784] (layout-only, like the baseline's host-side output untranspose), so
    the kernel does no PE transposes and no DVE layout copies.
  * The whole conv path runs in bf16: weights/inputs/gateBias are cast on
    the host, conv = 36 shifted bf16 matmuls + a bias identity-matmul per
    392-pixel window accumulating in fp32 PSUM. bf16 enables the PE's Fast
    Weight Load path (fp32r disables it), roughly halving per-matmul cost.
  * gapI never touches the conv output: by linearity 784*gapI is a
    combination of 9 masked pixel sums of the raw input with host-folded
    A-matrices (sums of conv taps). The masked sums (full/edge/corner) are
    free-dim DVE reductions over the staged channel-major tiles, so the
    AllGather of gate drivers fires ~45us in, long before the first
    elementwise consumer.
  * The (28*b+h)%64 gate scrambling makes gates cross-batch: cores AllGather
    their local GAP columns and a per-core index-array input drives an
    indirect-DMA gather of exactly the gate rows this core needs (the SPMD
    program stays identical across cores; only input data differs).
"""
import numpy as np
import ml_dtypes

import bass_rust
import concourse.bass as bass
import concourse.mybir as mybir
import concourse.tile as tile
from concourse.bass_utils import run_bass_kernel_spmd

F32 = mybir.dt.float32
BF16 = mybir.dt.bfloat16
I32 = mybir.dt.int32
AF = mybir.ActivationFunctionType
ALU = mybir.AluOpType

N_CORES = 8
B, H, W, CIN, D = 64, 28, 28, 256, 256
BL = B // N_CORES          # local batches per core
PIX = H * W                # 784
PAD = 30                   # padded row/col length
XTLEN = PAD * PAD          # 900
NW = 2                     # windows per batch
WROWS = H // NW            # 14
WN = WROWS * W             # 392
NCC = CIN // 128           # 2 input-channel chunks
NDC = D // 128             # 2 output-channel chunks

# tap order t = 3*kh + kw ; dh = kh-1, dw = kw-1
TAPS = [(kh, kw) for kh in range(3) for kw in range(3)]

# ---------------------------------------------------------------------------
# walrus fixup: split semaphore waits that exceed the per-instruction budget
MAX_WAITS = 1


def _split_excess_sem_waits(nc):
    counter = [0]
    for fn in nc.m.functions:
        for bb in fn.blocks:
            insts = bb.instructions
            i = 0
            while i < len(insts):
                inst = insts[i]
                si = inst.sync_info
                if si is not None and si.on_wait and len(si.on_wait) > MAX_WAITS:
                    waits = list(si.on_wait)
                    excess = waits[:-MAX_WAITS]
                    keep = waits[-MAX_WAITS:]
                    new_insts = []
                    for j in range(0, len(excess), MAX_WAITS):
                        chunk = excess[j:j + MAX_WAITS]
                        noop = mybir.InstNoOp(
                            name=f"I-waitsplit-{counter[0]}", ins=[], outs=[])
                        counter[0] += 1
                        noop.engine = inst.engine
                        noop.sync_info = bass_rust.SyncInfo(
                            on_wait=chunk, on_update=[])
                        nc.register_instruction(noop)
                        new_insts.append(noop)
                    inst.sync_info = bass_rust.SyncInfo(
                        on_wait=keep, on_update=list(si.on_update))
                    insts[i:i] = new_insts
                    i += len(new_insts)
                i += 1
    return nc


# ---------------------------------------------------------------------------
def _r3(ap, w):
    """view flat free dim as (rows, w)"""
    return ap.rearrange("c (r w) -> c r w", w=w)


def build_nc():
    nc = bass.Bass("TRN2", target_bir_lowering=False, debug=False,
                   num_devices=N_CORES)

    dram = {}
    dram["xin"] = nc.dram_tensor("inputs_t", [BL, NCC, 128, XTLEN], BF16,
                                 kind="ExternalInput").ap()
    dram["xhid"] = nc.dram_tensor("hidden_t", [BL, NCC, 128, XTLEN], BF16,
                                  kind="ExternalInput").ap()
    dram["xsp"] = nc.dram_tensor("state_t", [BL, NCC, 128, PIX], F32,
                                 kind="ExternalInput").ap()
    dram["w1"] = nc.dram_tensor("w1_bf", [9, NCC, 128, D], BF16,
                                kind="ExternalInput").ap()
    dram["w2"] = nc.dram_tensor("w2_bf", [9, NCC, 128, D], BF16,
                                kind="ExternalInput").ap()
    dram["afold"] = nc.dram_tensor("a_fold", [NCC, 9, 128, D], BF16,
                                   kind="ExternalInput").ap()
    dram["gb"] = nc.dram_tensor("gbias_t", [NDC, 128, PIX], F32,
                                kind="ExternalInput").ap()
    dram["vec"] = {}
    for nm in ("wxi", "whi", "inputBias", "wxf", "whf", "forgetBias",
               "wxo", "who", "outputBias"):
        dram["vec"][nm] = nc.dram_tensor(nm, [128, NDC], F32,
                                         kind="ExternalInput").ap()
    dram["ident"] = nc.dram_tensor("identity", [128, 128], F32,
                                   kind="ExternalInput").ap()
    dram["idx"] = nc.dram_tensor("gate_idx", [4, 224, 1], I32,
                                 kind="ExternalInput").ap()
    dram["hidden"] = nc.dram_tensor("hidden", [BL, NDC, 128, PIX], F32,
                                    kind="ExternalOutput").ap()
    dram["state"] = nc.dram_tensor("state", [BL, NDC, 128, PIX], F32,
                                   kind="ExternalOutput").ap()
    dram["cc_in"] = nc.dram_tensor("cc_in", [32, 128], F32, kind="Internal").ap()
    dram["cc_out"] = nc.dram_tensor("cc_out", [N_CORES * 32, 128], F32,
                                    kind="Internal", addr_space="Shared").ap()

    ctx_mgr = nc.allow_low_precision("bf16 conv path")
    ctx_mgr.__enter__()
    with tile.TileContext(nc) as tc:
        _build_body(nc, tc, dram)
    ctx_mgr.__exit__(None, None, None)
    return nc


def _build_body(nc, tc, dram):
    from contextlib import ExitStack
    ctx = ExitStack()
    pool = lambda **kw: ctx.enter_context(tc.tile_pool(**kw))

    const = pool(name="const", bufs=1)
    xspp = pool(name="xspp", bufs=6)       # [128, 784] f32 state tiles
    gtp = pool(name="gtp", bufs=14)        # [128, 392] f32 tanh-conv tiles
    outb = pool(name="outb", bufs=4)       # [128, 784] f32 out tiles (per tag)
    ew = pool(name="ew", bufs=3)           # [128, 392] f32 scratch (per tag)
    gtmp = pool(name="gtmp", bufs=2)
    gath = pool(name="gath", bufs=2)
    ps_conv = pool(name="ps_conv", bufs=6, space="PSUM")
    ps_gap = pool(name="ps_gap", bufs=1, space="PSUM")
    ps_tr = pool(name="ps_tr", bufs=1, space="PSUM")

    # ---- constants; weights split across both DMA rings so conv(0) can
    # start ~8us in: w1 leads the sync ring, w2 follows gbias on scalar ----
    wc = [const.tile([128, 9 * NCC * D], BF16, tag=f"wc{conv}",
                     name=f"wc{conv}") for conv in range(2)]

    def wblk(conv, t, cc):
        off = (t * NCC + cc) * D
        return wc[conv][:, off:off + D]

    # dram [9,NCC,128,D] -> SBUF [128, (9,NCC,D)] in one strided DMA
    nc.sync.dma_start(
        wc[0][:].rearrange("p (t c n) -> p t c n", t=9, c=NCC),
        dram["w1"][:].rearrange("t c p n -> p t c n"))

    gbias = [const.tile([128, PIX], F32, tag=f"gbias{dc}", name=f"gbias{dc}")
             for dc in range(NDC)]
    for dc in range(NDC):
        nc.scalar.dma_start(gbias[dc][:], dram["gb"][dc])

    nc.scalar.dma_start(
        wc[1][:].rearrange("p (t c n) -> p t c n", t=9, c=NCC),
        dram["w2"][:].rearrange("t c p n -> p t c n"))

    afold = const.tile([128, NCC * 9 * D], BF16, tag="afold")

    def ablk(cc, g):
        off = (cc * 9 + g) * D
        return afold[:, off:off + D]

    nc.scalar.dma_start(
        afold[:].rearrange("p (c g n) -> p c g n", c=NCC, g=9),
        dram["afold"][:].rearrange("c g p n -> p c g n"))

    idx_sb = []
    for g4 in range(4):
        halves = []
        for hf in range(2):
            t = const.tile([112, 1], I32, tag=f"idx{g4}_{hf}")
            nc.scalar.dma_start(t[:], dram["idx"][g4, hf * 112:(hf + 1) * 112, :])
            halves.append(t)
        idx_sb.append(halves)

    vecs = {}
    for nm in dram["vec"]:
        t = const.tile([128, NDC], F32, tag=f"vec_{nm}")
        nc.scalar.dma_start(t[:], dram["vec"][nm][:])
        vecs[nm] = t

    ident = const.tile([128, 128], F32, tag="ident")
    nc.scalar.dma_start(ident[:], dram["ident"][:])

    # ---- padded input tiles (host pre-pads the zero borders) ----
    xt = {}       # xt[(j, tensor, cc)] -> [128, 900] bf16
    for j in range(BL):
        for tn in ("in", "hid"):
            for cc in range(NCC):
                xt[(j, tn, cc)] = const.tile(
                    [128, XTLEN], BF16, tag=f"x{tn}{j}_{cc}",
                    name=f"x{tn}{j}_{cc}")

    # masked-sum accumulators (columns written per batch)
    rawI = [const.tile([128, 9 * BL], BF16, tag=f"rawI{cc}", name=f"rawI{cc}")
            for cc in range(NCC)]
    rawH = [const.tile([128, BL], F32, tag=f"rawH{cc}", name=f"rawH{cc}")
            for cc in range(NCC)]

    # ---- per-batch input loading (sync/SP DMA ring) ----
    def emit_load(j):
        for tn, dsrc in (("in", dram["xin"]), ("hid", dram["xhid"])):
            for cc in range(NCC):
                nc.sync.dma_start(xt[(j, tn, cc)][:], dsrc[j, cc])

    xsp_tiles = {}

    def emit_load_state(j):
        ts = []
        for cc in range(NCC):
            s = xspp.tile([128, PIX], F32, tag="xsp", name=f"xsp{j}_{cc}")
            nc.sync.dma_start(s[:], dram["xsp"][j, cc])
            ts.append(s)
        xsp_tiles[j] = ts

    # ---- stage: masked-sum reductions (vector) over the padded tiles ----
    def emit_stage(j):
        for tn in ("in", "hid"):
            for cc in range(NCC):
                s = xt[(j, tn, cc)]
                s3 = _r3(s[:], PAD)                     # [128, 30, 30]
                if tn == "hid":
                    # gapH: full pixel sum only (borders are zero)
                    nc.vector.tensor_reduce(
                        rawH[cc][:, j:j + 1], s[:], mybir.AxisListType.X,
                        ALU.add)
                else:
                    rv = rawI[cc][:].rearrange("c (g b) -> c g b", b=BL)
                    # group order: S, Rf(row0), Rl(row27), Cf(col0),
                    # Cl(col27), K00, K0L, KL0, KLL
                    nc.vector.tensor_reduce(
                        rv[:, 0, j:j + 1], s[:], mybir.AxisListType.X, ALU.add)
                    nc.vector.tensor_reduce(
                        rv[:, 1, j:j + 1], s[:, PAD + 1:PAD + 1 + W],
                        mybir.AxisListType.X, ALU.add)
                    nc.vector.tensor_reduce(
                        rv[:, 2, j:j + 1], s[:, 28 * PAD + 1:28 * PAD + 1 + W],
                        mybir.AxisListType.X, ALU.add)
                    nc.vector.tensor_reduce(
                        rv[:, 3, j:j + 1], s3[:, 1:29, 1:2],
                        mybir.AxisListType.XY, ALU.add)
                    nc.vector.tensor_reduce(
                        rv[:, 4, j:j + 1], s3[:, 1:29, 28:29],
                        mybir.AxisListType.XY, ALU.add)
                    corners = s3[:, 1:29:27, 1:29:27]   # [128, 2, 2]
                    dstc = rv[:, 5:9, j:j + 1].rearrange(
                        "c (x y) o -> c x (y o)", x=2)
                    nc.vector.tensor_copy(dstc, corners)

    # ---- conv windows ----
    gt_tiles = {}

    def emit_conv(j):
        for wi in range(NW):
            for dc in range(NDC):
                h0 = 1 + wi * WROWS
                base = (h0 - 1) * W
                p = ps_conv.tile([128, WN], F32, tag="pconv", name="pconv")
                p3 = _r3(p[:], W)
                first = True
                for conv, tn in ((0, "in"), (1, "hid")):
                    for t, (kh, kw) in enumerate(TAPS):
                        dh, dwid = kh - 1, kw - 1
                        for cc in range(NCC):
                            rhs = _r3(xt[(j, tn, cc)][:], PAD)[
                                :, h0 + dh:h0 + dh + WROWS,
                                1 + dwid:1 + dwid + W]
                            last = (conv == 1 and t == 8 and cc == NCC - 1)
                            nc.tensor.matmul(
                                p3, wblk(conv, t, cc)[:, dc * 128:(dc + 1) * 128],
                                rhs, start=first, stop=last)
                            first = False
                nc.vector.tensor_tensor(out=p[:], in0=p[:],
                                        in1=gbias[dc][:, base:base + WN],
                                        op=ALU.add)
                gt = gtp.tile([128, WN], F32, tag="gt", name="gt")
                nc.scalar.activation(gt[:], p[:], AF.Tanh)
                gt_tiles[(j, wi, dc)] = gt

    # ---- gap combine + AllGather ----
    def emit_combine():
        gap_ps = ps_gap.tile([8, D], F32, tag="gapI")
        for cc in range(NCC):
            rv = rawI[cc][:].rearrange("c (g b) -> c g b", b=BL)
            for g in range(9):
                nc.tensor.matmul(gap_ps[:], rv[:, g], ablk(cc, g),
                                 start=(cc == 0 and g == 0),
                                 stop=(cc == NCC - 1 and g == 8))
        gapI_sb = const.tile([8, D], F32, tag="gapI_sb")
        nc.vector.tensor_copy(gapI_sb[:], gap_ps[:])
        nc.scalar.dma_start(dram["cc_in"][0:8, :], gapI_sb[:, 0:128])
        nc.scalar.dma_start(dram["cc_in"][8:16, :], gapI_sb[:, 128:256])
        for cc in range(NCC):
            pt = ps_tr.tile([128, 128], F32, tag="ptr", name="pt_gapH")
            pt = pt[0:8, :]
            nc.tensor.transpose(pt, rawH[cc][:], ident[:])
            hs = const.tile([8, 128], F32, tag=f"gapH_sb{cc}",
                            name=f"gapHsb{cc}")
            nc.vector.tensor_copy(hs[:], pt)
            nc.scalar.dma_start(dram["cc_in"][16 + 8 * cc:24 + 8 * cc, :],
                                hs[:])
        nc.gpsimd.collective_compute(
            "AllGather", ALU.bypass, replica_groups=[list(range(N_CORES))],
            ins=[dram["cc_in"][:]], outs=[dram["cc_out"][:]])

    # ---- gather + gate tables ----
    gates = {}

    def emit_gates():
        sel = [const.tile([128, 224], F32, tag=f"sel{g4}", name=f"sel{g4}")
               for g4 in range(4)]
        for g4 in range(4):
            for hf in range(2):
                gtile = gath.tile([112, 128], F32, tag="gath", name="gath")
                nc.gpsimd.indirect_dma_start(
                    out=gtile[:], out_offset=None, in_=dram["cc_out"][:],
                    in_offset=bass.IndirectOffsetOnAxis(
                        ap=idx_sb[g4][hf][:, :1], axis=0))
                pt = ps_tr.tile([128, 128], F32, tag="ptr", name="pt_gath")
                nc.tensor.transpose(pt[:, 0:112], gtile[:],
                                    ident[0:112, 0:112])
                nc.vector.tensor_copy(sel[g4][:, hf * 112:(hf + 1) * 112],
                                      pt[:, 0:112])
        for gate, wx, wh, bi in (("i", "wxi", "whi", "inputBias"),
                                 ("f", "wxf", "whf", "forgetBias"),
                                 ("o", "wxo", "who", "outputBias")):
            per_dc = []
            for dc in range(NDC):
                t1 = gtmp.tile([128, 224], F32, tag="gm1", name="gm1")
                nc.vector.tensor_scalar_mul(t1[:], sel[dc][:],
                                            vecs[wx][:, dc:dc + 1])
                t2 = gtmp.tile([128, 224], F32, tag="gm2", name="gm2")
                nc.vector.tensor_scalar_mul(t2[:], sel[2 + dc][:],
                                            vecs[wh][:, dc:dc + 1])
                nc.vector.tensor_tensor(out=t1[:], in0=t1[:], in1=t2[:],
                                        op=ALU.add)
                gt = const.tile([128, 224], F32, tag=f"gate_{gate}{dc}",
                                name=f"gate_{gate}{dc}")
                nc.scalar.activation(gt[:], t1[:], AF.Sigmoid,
                                     bias=vecs[bi][:, dc:dc + 1])
                per_dc.append(gt)
            gates[gate] = per_dc

    # ---- elementwise + store ----
    def emit_ew(j):
        stT = [outb.tile([128, PIX], F32, tag="stT", name=f"stT{j}_{dc}")
               for dc in range(NDC)]
        hidT = [outb.tile([128, PIX], F32, tag="hidT", name=f"hidT{j}_{dc}")
                for dc in range(NDC)]
        for wi in range(NW):
            for dc in range(NDC):
                h0 = 1 + wi * WROWS
                base = (h0 - 1) * W
                t0 = j * H + (h0 - 1)

                def gw(gate):
                    return gates[gate][dc][:, t0:t0 + WROWS].to_broadcast(
                        [128, WROWS, W])

                gt = gt_tiles.pop((j, wi, dc))
                sp3 = _r3(xsp_tiles[j][dc][:, base:base + WN], W)
                g3 = _r3(gt[:], W)
                st3 = _r3(stT[dc][:, base:base + WN], W)
                hd3 = _r3(hidT[dc][:, base:base + WN], W)
                s1 = ew.tile([128, WN], F32, tag="s1", name="s1")
                nc.gpsimd.tensor_tensor(out=_r3(s1[:], W), in0=sp3,
                                        in1=gw("f"), op=ALU.mult)
                s2 = ew.tile([128, WN], F32, tag="s2", name="s2")
                nc.gpsimd.tensor_tensor(out=_r3(s2[:], W), in0=g3,
                                        in1=gw("i"), op=ALU.mult)
                nc.vector.tensor_tensor(out=st3, in0=_r3(s1[:], W),
                                        in1=_r3(s2[:], W), op=ALU.add)
                th = ew.tile([128, WN], F32, tag="th", name="th")
                nc.scalar.activation(th[:], stT[dc][:, base:base + WN],
                                     AF.Tanh)
                nc.gpsimd.tensor_tensor(out=hd3, in0=_r3(th[:], W),
                                        in1=gw("o"), op=ALU.mult)
        for dname, buf in (("state", stT), ("hidden", hidT)):
            for dc in range(NDC):
                nc.scalar.dma_start(dram[dname][j, dc], buf[dc][:])

    # ================= schedule =================
    for j in range(BL):
        emit_load(j)
    for j in range(BL):
        emit_load_state(j)
    for j in range(BL):
        emit_stage(j)
    emit_conv(0)
    emit_combine()
    emit_conv(1)
    emit_conv(2)
    emit_gates()
    emit_conv(3)
    emit_ew(0)
    for j in range(4, BL):
        emit_conv(j)
        emit_ew(j - 3)
    for j in range(BL - 3, BL):
        emit_ew(j)

    ctx.close()


# ---------------------------------------------------------------------------
_NC_CACHE = None


def _get_nc():
    global _NC_CACHE
    if _NC_CACHE is None:
        nc = build_nc()
        _split_excess_sem_waits(nc)
        _NC_CACHE = nc
    return _NC_CACHE


def _gate_idx(core):
    idx = np.empty((4, 224, 1), np.int32)
    for j in range(BL):
        for hh in range(H):
            t = j * H + hh
            sel_b = (H * (BL * core + j) + hh) % B
            cp, bp = sel_b // BL, sel_b % BL
            for g in range(4):
                idx[g, t, 0] = cp * 32 + g * 8 + bp
    return idx


def _make_in_maps(inputs):
    f32 = np.float32
    bf16 = ml_dtypes.bfloat16

    w1 = np.ascontiguousarray(inputs["wconvInput"], dtype=f32)  # [3,3,CIN,D]
    w2 = np.ascontiguousarray(inputs["wconvHidden"], dtype=f32)
    # w{1,2}_bf[t, cc, 128, D]
    w1b = np.empty((9, NCC, 128, D), dtype=bf16)
    w2b = np.empty((9, NCC, 128, D), dtype=bf16)
    for wb, w in ((w1b, w1), (w2b, w2)):
        for t, (kh, kw) in enumerate(TAPS):
            for cc in range(NCC):
                wb[t, cc] = w[kh, kw, cc * 128:(cc + 1) * 128, :]

    # A-fold for gapI: 784*gapI = sum_g raw_g^T @ A_g  (group order
    # S, Rf, Rl, Cf, Cl, K00, K0L, KL0, KLL; edge groups negated)
    wt = w1.reshape(9, CIN, D)
    A = np.empty((9, CIN, D), f32)
    A[0] = wt.sum(0)
    A[1] = -(wt[6] + wt[7] + wt[8])
    A[2] = -(wt[0] + wt[1] + wt[2])
    A[3] = -(wt[2] + wt[5] + wt[8])
    A[4] = -(wt[0] + wt[3] + wt[6])
    A[5], A[6], A[7], A[8] = wt[8], wt[6], wt[2], wt[0]
    afold = np.empty((NCC, 9, 128, D), dtype=bf16)
    for cc in range(NCC):
        afold[cc] = A[:, cc * 128:(cc + 1) * 128, :]

    gb = np.ascontiguousarray(inputs["gateBias"], dtype=f32).reshape(PIX, D)
    gbias_t = np.ascontiguousarray(gb.T.reshape(NDC, 128, PIX))

    shared = {
        "w1_bf": w1b,
        "w2_bf": w2b,
        "a_fold": afold,
        "gbias_t": gbias_t,
        "identity": np.eye(128, dtype=f32),
    }
    for nm in ("wxi", "whi", "inputBias", "wxf", "whf", "forgetBias",
               "wxo", "who", "outputBias"):
        v = np.ascontiguousarray(inputs[nm], dtype=f32).reshape(D)
        if nm.startswith("wx") or nm.startswith("wh"):
            v = v / PIX
        shared[nm] = np.ascontiguousarray(v.reshape(NDC, 128).T)  # [128, NDC]

    def chan_major(x, dtype):
        # [B, PIX, C] -> [B, NCC, 128, PIX]
        xt = np.ascontiguousarray(x.reshape(B, PIX, CIN).transpose(0, 2, 1))
        return xt.reshape(B, NCC, 128, PIX).astype(dtype)

    def chan_major_padded(x):
        # [B, PIX, C] -> [B, NCC, 128, 30*30] bf16 with zero borders
        cm = chan_major(x, bf16).reshape(B, NCC, 128, H, W)
        out = np.zeros((B, NCC, 128, PAD, PAD), dtype=bf16)
        out[:, :, :, 1:29, 1:29] = cm
        return out.reshape(B, NCC, 128, XTLEN)

    xin = chan_major_padded(np.asarray(inputs["inputs"], dtype=f32))
    xhp = chan_major_padded(np.asarray(inputs["hidden_prev"], dtype=f32))
    xsp = chan_major(np.asarray(inputs["state_prev"], dtype=f32), f32)

    in_maps = []
    for k in range(N_CORES):
        sl = slice(k * BL, (k + 1) * BL)
        m = dict(shared)
        m["inputs_t"] = xin[sl]
        m["hidden_t"] = xhp[sl]
        m["state_t"] = xsp[sl]
        m["gate_idx"] = _gate_idx(k)
        in_maps.append(m)
    return in_maps


def kernel(**inputs):
    nc = _get_nc()
    in_maps = _make_in_maps(inputs)
    res = run_bass_kernel_spmd(nc, in_maps, core_ids=list(range(N_CORES)))

    def unshard(name):
        # per-core outputs are [BL, NDC, 128, PIX] (channel-major)
        full = np.concatenate([res.results[k][name] for k in range(N_CORES)],
                              axis=0)
        return np.ascontiguousarray(full.transpose(0, 3, 1, 2)).reshape(
            B, H, W, D)

    return unshard("hidden"), unshard("state")
